# revision 57
# baseline (speedup 1.0000x reference)
"""AttentionBlock kernel for 8 Trainium2 NeuronCores (v10, 255us HW).

Problem (hardcoded shapes): x [4, 256, 64, 64] f32.
  GroupNorm(32 groups) -> qkv 1x1 conv (768x256) -> 4-head attention over
  n=4096 tokens (hd=64) -> proj 1x1 conv -> residual add.

Sharding: 8 cores = (batch b in 0..3) x (query-half in 0..1).  Each core
computes GroupNorm + K/V for its whole batch image (duplicated across the
2 cores of a batch, cheap) and attention + proj + residual for its half of
the query positions (2048 of 4096).  Key order is permuted so the core's
query slice comes first; softmax is permutation-invariant over keys.

v10 structure (vs the 327us v4 baseline; all deltas HW-measured):
  - fp8e4m3 DoubleRow PV matmuls: each PV covers a chunk PAIR (contraction
    256 = 2 fp8 weights/PE cell, 0.5 cyc/col) -- halves the PV side of the
    PE stream, which paces the steady state.  P comes from ScalarE exact
    exp (even chunks, fp8 out) / VectorE uint8 Schraudolph (odd chunks);
    exp args are shifted by SHIFT8 (cancels in softmax) to center weights
    in e4m3 range.  V is cast to fp8 at drain time.
  - fp8 DoubleRow qkv projections too (x8/wt8 operands; GroupNorm stats
    still from the bf16 copy; GroupNorm affine folded into wt8 on-chip).
  - k-bias dropped entirely: q.(Wk x + bk) adds a per-query constant to
    scores, which softmax cancels.  k2 drains are pure ScalarE copies,
    balancing jp0's drain load across both act engines.
  - HAM management: the PE clock gate starts throttled (1.2 vs 2.4 GHz)
    and re-throttles after idle windows.  Dummy matmuls (free-running +
    bn_stats-paced) keep the PE busy through the GroupNorm prologue; the
    steady stream then holds one continuous ~200us warm window.
  - GroupNorm rstd via fast-inverse-sqrt + 1 Newton step on the DVE: the
    Exp activation table is the only set ScalarE ever loads (a Ln<->Exp
    switch costs 1.3us and thrashed per-t when scheduled naively).
  - emit_proj deferred to cyc 14 of the NEXT pair: the att division chain
    (drain -> DMA spread -> recip -> DRAM-bounce broadcast -> GpSimd mul)
    is ~10us of DMA latency; proj matmuls emitted earlier dam up the
    in-order PE stream (and the P-tile pool rotation the exps wait on),
    which cost ~6.5us + a HAM re-throttle per j boundary.
  - Softmax exp split ~16/16 between ScalarE (exact) and VectorE
    (Schraudolph); both land ~88% busy, just under the PE pace.
"""

import sys

import numpy as np

sys.path.insert(0, "/opt/trn_rl_repo")

B, C, HW = 4, 256, 4096
NQ = HW // 2  # queries per core
NH, HD = 4, 64
G = 32  # groups
EPS = 1e-5

# Per-(j,pair) exp split: which key chunks (0..31) VectorE handles.
# jp = 2*j + pair. Early jps keep VectorE light (it also drains the qkv
# projections); never the last chunks (they gate the psO drain).
# NOTE: pool placement (psA vs psD) is fixed by kc % 3 == 2 regardless of
# engine -- the lag-2 PE pipeline keeps 3 score tiles live, which exactly
# fits psA bufs=2 + psD bufs=1 when psD takes every third chunk.
# Engine pattern alternates so neither engine ever gets two consecutive
# chunks (a 2:1 run structure makes the faster engine the 765ns/chunk
# pacer); chunk 31 stays on ScalarE so the psO drain is not queued
# behind VectorE exps.
DVE_SCHED = {
    0: (1, 3, 7, 9, 13, 15, 19, 21, 25, 27, 29),
    1: (1, 3, 5, 7, 9, 11, 13, 15, 19, 21, 23, 25, 27, 29),
}
DVE_STEADY = (0, 2, 4, 6, 8, 10, 12, 14, 16, 18, 20, 22, 24, 26, 28, 30)

# Schraudolph constants, fp8e4m3 flavor: byte = trunc(x_raw * A8 + B8)
# approximates exp(x_raw/8 - SHIFT8) as e4m3 bits.  The softmax is
# shift-invariant, so SHIFT8 cancels exactly (the ones-column denominator
# is scaled identically); it just centers the weights in e4m3's range so
# almost nothing is subnormal or clamped.  Numerically validated: max
# softmax-output error ~5e-3 (vs out scale 0.1) with fp8 V, ~2.4x the old
# bf16 trick, still ~10x inside the rel-err gate.
LOG2E = 1.4426950408889634
SHIFT8 = 2.0
A8 = LOG2E
B8 = 56.0 - 8.0 * SHIFT8 * LOG2E

# HAM warmup dummy-matmul counts (see _build): free-running batch from
# t~6.5us, then a few per bn_stats completion to pace through the prologue.
N_WARM_FREE = 14
N_WARM_PER_STAT = 3

_CACHE = {}


def _build():
    import concourse.bass as bass
    import concourse.tile as tile
    from concourse import bacc, mybir
    from concourse.tile import add_dep_helper

    f32 = mybir.dt.float32
    bf16 = mybir.dt.bfloat16
    u8 = mybir.dt.uint8
    f8 = mybir.dt.float8e4
    AF = mybir.ActivationFunctionType
    ALU = mybir.AluOpType
    DR = mybir.MatmulPerfMode.DoubleRow

    nc = bacc.Bacc(
        "TRN2",
        target_bir_lowering=False,
        debug=False,
        enable_asserts=False,
        num_devices=8,
    )

    x_d = nc.dram_tensor("x", [C, HW], f32, kind="ExternalInput").ap()
    x_bf_d = nc.dram_tensor("x_bf", [C, HW], bf16, kind="ExternalInput").ap()
    x_f8_d = nc.dram_tensor("x_f8", [C, HW], f8, kind="ExternalInput").ap()
    qkv_wt_d = nc.dram_tensor("qkv_wt", [C, 3 * C], bf16, kind="ExternalInput").ap()
    qkv_b_d = nc.dram_tensor("qkv_b", [3 * C], f32, kind="ExternalInput").ap()
    proj_wt_d = nc.dram_tensor("proj_wt", [C, C], bf16, kind="ExternalInput").ap()
    proj_b_d = nc.dram_tensor("proj_b", [C], f32, kind="ExternalInput").ap()
    gn_w_d = nc.dram_tensor("gn_w", [C], f32, kind="ExternalInput").ap()
    gn_b_d = nc.dram_tensor("gn_b", [C], f32, kind="ExternalInput").ap()
    sel_d = nc.dram_tensor("sel", [128, 16], f32, kind="ExternalInput").ap()
    selT_d = nc.dram_tensor("selT", [16, 128], f32, kind="ExternalInput").ap()
    y_d = nc.dram_tensor("y", [C, NQ], f32, kind="ExternalOutput").ap()

    x_r = x_d.rearrange("(t p) n -> p t n", p=128)  # c = t*128 + p
    x_bf_r = x_bf_d.rearrange("(t p) n -> p t n", p=128)
    x_f8_r = x_f8_d.rearrange("(t p) n -> p t n", p=128)
    y_r = y_d.rearrange("(t p) n -> p t n", p=128)

    with tile.TileContext(nc) as tc:
        with (
            tc.tile_pool(name="const", bufs=1) as const,
            tc.tile_pool(name="big", bufs=1) as big,
            tc.tile_pool(name="work", bufs=2) as work,
            tc.tile_pool(name="pf8", bufs=6) as pf8,
            tc.tile_pool(name="psA", bufs=2, space="PSUM") as psA_pool,
            tc.tile_pool(name="psD", bufs=1, space="PSUM") as psD_pool,
            tc.tile_pool(name="psO", bufs=2, space="PSUM") as psO_pool,
            tc.tile_pool(name="dram", bufs=2, space="DRAM") as dram_pool,
        ):
            # ---- x load FIRST: everything gates on GroupNorm stats over the
            # full image, and each DMA costs ~600ns of issue time on the Sync
            # queue -- so the x transfers go ahead of all const DMAs.  Split
            # into 8 chunks so bn_stats can start on the first quarter while
            # the rest streams in.
            xb = big.tile([128, 2, HW], bf16, tag="xb")
            for t in range(2):
                for q4 in range(4):
                    nc.sync.dma_start(
                        xb[:, t, q4 * 1024 : (q4 + 1) * 1024],
                        x_bf_r[:, t, q4 * 1024 : (q4 + 1) * 1024],
                    )
            # fp8 copy of x for the qkv DoubleRow matmuls (bf16 copy above
            # still feeds the GroupNorm stats); needed from ~24us on.
            x8 = big.tile([128, 2, HW], f8, tag="x8")
            for t in range(2):
                nc.sync.dma_start(x8[:, t, :], x_f8_r[:, t, :])

            # ---- HAM warmup + activation-table preload ----
            # The PE clock gate (HAM) only un-throttles (1.2 -> 2.4 GHz)
            # after ~3.4us of sustained matmul activity, and re-throttles
            # after an idle window.  The GroupNorm-stats prologue leaves the
            # PE idle for ~15us, so the whole first attention stream used to
            # run at half clock.  Dummy matmuls (results never read) keep the
            # PE busy from t~6.5us until the real stream starts: a free-run
            # batch first, then batches paced by bn_stats completions so they
            # stretch to the end of the stats chain without delaying it.
            warm = const.tile([128, 512], bf16, tag="warm")
            nc.vector.memset(warm, 0.25)

            def dummy_mm(dep=None):
                # shares the psO tag so the pool stays at 2 PSUM banks; the
                # warmup rotation retires long before the first real psO.
                psW = psO_pool.tile([128, 512], f32, tag="psO")
                m = nc.tensor.matmul(
                    psW, lhsT=warm[:, 0:128], rhs=warm, start=True, stop=True
                )
                if dep is not None:
                    add_dep_helper(m.ins, dep.ins, sync=True, reason="warm pace")

            # The Exp activation table loads on first use (~1.3us); trigger
            # the load now while ScalarE is idle instead of on the GroupNorm
            # critical path.  Exp is the ONLY table set ever used (rstd is
            # computed on the DVE below), so there is no table switching.
            tpre = const.tile([1, 8], f32, tag="tpre")
            nc.vector.memset(tpre, 1.0)
            nc.scalar.activation(tpre, tpre, AF.Exp)
            # per-partition bias tile for the fp8 exp shift
            shift_sb = const.tile([128, 1], f32, tag="shift8")
            nc.vector.memset(shift_sb, -SHIFT8)

            for _ in range(N_WARM_FREE):
                dummy_mm()

            # ---- constants / weights ----
            wt_sb = const.tile([128, 2, 3 * C], bf16, tag="wt")
            nc.sync.dma_start(wt_sb, qkv_wt_d.rearrange("(t p) o -> p t o", p=128))
            wproj_sb = const.tile([64, NH, C], bf16, tag="wproj")
            nc.sync.dma_start(wproj_sb, proj_wt_d.rearrange("(h p) o -> p h o", p=64))
            qkvb_sb = const.tile([128, 6], f32, tag="qkvb")
            nc.sync.dma_start(qkvb_sb, qkv_b_d.rearrange("(s p) -> p s", p=128))
            vb_sb = const.tile([64, NH], f32, tag="vb")
            nc.sync.dma_start(vb_sb, qkv_b_d[2 * C :].rearrange("(h p) -> p h", p=64))
            projb_sb = const.tile([128, 2], f32, tag="projb")
            nc.sync.dma_start(projb_sb, proj_b_d.rearrange("(t p) -> p t", p=128))
            gnw_sb = const.tile([128, 2], f32, tag="gnw")
            nc.sync.dma_start(gnw_sb, gn_w_d.rearrange("(t p) -> p t", p=128))
            gnb_sb = const.tile([128, 2], f32, tag="gnb")
            nc.sync.dma_start(gnb_sb, gn_b_d.rearrange("(t p) -> p t", p=128))

            # group-selector matrices (channels<->groups), used for the tiny
            # cross-partition reductions in GroupNorm stats.
            ones1 = const.tile([1, 64], f32, tag="ones1")
            nc.vector.memset(ones1, 1.0)
            sel = const.tile([128, 16], f32, tag="sel")  # sel[p, g]=1 if p//8==g
            nc.sync.dma_start(sel, sel_d)
            selT = const.tile([16, 128], f32, tag="selT")
            nc.sync.dma_start(selT, selT_d)

            # ---- GroupNorm stats (bn_stats free dim is capped at 512) ----
            stats = const.tile([128, 2, 8, 6], f32, tag="stats")
            mv = const.tile([128, 2, 2], f32, tag="mv")
            st_handles = []
            for t in range(2):
                for jj in range(8):
                    st = nc.vector.bn_stats(
                        stats[:, t, jj, :], xb[:, t, jj * 512 : (jj + 1) * 512]
                    )
                    st_handles.append(st)
                nc.vector.bn_aggr(mv[:, t, :], stats[:, t])
            # paced warmup: a few dummy matmuls gated on each bn_stats so the
            # PE never idles past the HAM re-throttle window while the DVE
            # stats chain (the prologue critical path) runs.
            for st in st_handles:
                for _ in range(N_WARM_PER_STAT):
                    dummy_mm(dep=st)
            # me2[:, t, 0] = mean_c ; me2[:, t, 1] = E[x^2]_c = var + mean^2
            me2 = const.tile([128, 2, 2], f32, tag="me2")
            for t in range(2):
                nc.vector.tensor_copy(me2[:, t, 0:1], mv[:, t, 0:1])
                nc.vector.tensor_mul(me2[:, t, 1:2], mv[:, t, 0:1], mv[:, t, 0:1])
                nc.vector.tensor_add(me2[:, t, 1:2], me2[:, t, 1:2], mv[:, t, 1:2])
            # group sums via selector matmul (fp32 for exactness)
            psg = psA_pool.tile([16, 2, 2], f32, tag="psS")
            for t in range(2):
                nc.tensor.matmul(
                    psg[:, t, :], lhsT=sel, rhs=me2[:, t, :], start=True, stop=True
                )
            gsb = const.tile([16, 2, 2], f32, tag="gsb")
            gmr = const.tile([16, 2, 2], f32, tag="gmr")  # (mean_g, rstd_g)
            # rstd = rsqrt(var+eps) on the DVE via fast-inverse-sqrt + one
            # Newton step (rel err ~2e-3, noise vs the ~3% Schraudolph
            # ripple).  Keeping this off ScalarE means the Exp activation
            # table is the only set ever loaded -- the Ln<->Exp table
            # switches (1.3us each) this replaces sat on the critical path
            # and could thrash per-t.
            i32 = mybir.dt.int32
            magic = const.tile([16, 1], i32, tag="magic")
            nc.vector.memset(magic, 0x5F3759DF)
            rsq = const.tile([16, 2, 8], f32, tag="rsq")
            for t in range(2):
                nc.vector.tensor_scalar_mul(gsb[:, t, :], psg[:, t, :], 1.0 / 8.0)
                nc.vector.tensor_copy(gmr[:, t, 0:1], gsb[:, t, 0:1])
                # var_g = E2_g - mean_g^2
                nc.vector.tensor_mul(gmr[:, t, 1:2], gsb[:, t, 0:1], gsb[:, t, 0:1])
                nc.vector.tensor_sub(gmr[:, t, 1:2], gsb[:, t, 1:2], gmr[:, t, 1:2])
                ve = rsq[:, t, 0:1]  # var + eps
                nc.vector.tensor_scalar(
                    ve, gmr[:, t, 1:2], EPS, 0.0, ALU.add, ALU.bypass
                )
                # y0 bits = magic - (bits(ve) >> 1)
                sh = rsq[:, t, 1:2]
                nc.vector.tensor_scalar(
                    sh.bitcast(i32), ve.bitcast(i32), 1, 0,
                    ALU.logical_shift_right, ALU.bypass,
                )
                y0 = rsq[:, t, 2:3]
                nc.vector.tensor_sub(y0.bitcast(i32), magic, sh.bitcast(i32))
                # one Newton step: y1 = y0 * (1.5 - 0.5*ve*y0^2)
                aa = rsq[:, t, 3:4]
                nc.vector.tensor_mul(aa, y0, y0)
                nc.vector.tensor_mul(aa, aa, ve)
                nc.vector.tensor_scalar(aa, aa, -0.5, 1.5, ALU.mult, ALU.add)
                nc.vector.tensor_mul(gmr[:, t, 1:2], y0, aa)
            # broadcast group stats back to channels
            psb = psA_pool.tile([128, 2, 2], f32, tag="psS")
            for t in range(2):
                nc.tensor.matmul(
                    psb[:, t, :], lhsT=selT, rhs=gmr[:, t, :], start=True, stop=True
                )
            # per-channel affine: xn = x * a + bcoef  (xn in bf16 for PE)
            ab = const.tile([128, 2, 2], f32, tag="ab")
            for t in range(2):
                nc.vector.tensor_mul(ab[:, t, 0:1], psb[:, t, 1:2], gnw_sb[:, t : t + 1])
                nc.vector.tensor_mul(ab[:, t, 1:2], psb[:, t, 0:1], ab[:, t, 0:1])
                nc.vector.tensor_sub(ab[:, t, 1:2], gnb_sb[:, t : t + 1], ab[:, t, 1:2])
            # Fold the GroupNorm affine into the qkv weights instead of
            # normalizing x:  W(ax+b) + c = (Wa)x + (Wb + c).
            bvec = const.tile([128, 2, 1], bf16, tag="bvec")
            for t in range(2):
                nc.vector.tensor_copy(bvec[:, t, :], ab[:, t, 1:2])
            qkvb2 = const.tile([128, 6], f32, tag="qkvb2")
            for s in range(2):  # q 128-wide output sections (k bias cancels)
                psq = psA_pool.tile([128, 1], f32, tag="psS")
                for t in range(2):
                    nc.tensor.matmul(
                        psq,
                        lhsT=wt_sb[:, t, s * 128 : (s + 1) * 128],
                        rhs=bvec[:, t, :],
                        start=(t == 0),
                        stop=(t == 1),
                    )
                nc.vector.tensor_add(qkvb2[:, s : s + 1], qkvb_sb[:, s : s + 1], psq)
            vb2 = const.tile([64, NH], f32, tag="vb2")
            for h in range(NH):  # v bias per head (64-wide, base partition 0)
                psv = psA_pool.tile([64, 1], f32, tag="psS")
                for t in range(2):
                    nc.tensor.matmul(
                        psv,
                        lhsT=wt_sb[:, t, 2 * C + h * 64 : 2 * C + (h + 1) * 64],
                        rhs=bvec[:, t, :],
                        start=(t == 0),
                        stop=(t == 1),
                    )
                nc.vector.tensor_add(vb2[:, h : h + 1], vb_sb[:, h : h + 1], psv)
            # out = PV/denom + vb2 exactly, so the v bias folds into the proj
            # bias: projb2 = proj_b + proj_w @ vb2.
            vb2bf = const.tile([64, NH], bf16, tag="vb2bf")
            nc.vector.tensor_copy(vb2bf, vb2)
            projb2 = const.tile([128, 2], f32, tag="projb2")
            for ot in range(2):
                psB = psA_pool.tile([128, 1], f32, tag="psS")
                for h in range(NH):
                    nc.tensor.matmul(
                        psB,
                        lhsT=wproj_sb[:, h, ot * 128 : (ot + 1) * 128],
                        rhs=vb2bf[:, h : h + 1],
                        start=(h == 0),
                        stop=(h == 3),
                    )
                nc.vector.tensor_add(projb2[:, ot : ot + 1], projb_sb[:, ot : ot + 1], psB)
            # scale the weight columns in place (after the bias matmuls),
            # then cast the scaled section to fp8 for the DoubleRow qkv
            # matmuls (contraction 256 = both t-halves in one matmul at 0.5
            # cycles/column).  Only the pair-0 q/k sections are done up
            # front -- they gate the first score chunk; the rest is deferred
            # into the jp0 stream where the first exps provide cover.
            wt8 = const.tile([128, 2, 3 * C], f8, tag="wt8")

            def scale_sec(lo, hi):
                for t in range(2):
                    nc.vector.tensor_scalar_mul(
                        wt_sb[:, t, lo:hi], wt_sb[:, t, lo:hi], ab[:, t, 0:1]
                    )
                nc.vector.tensor_copy(wt8[:, :, lo:hi], wt_sb[:, :, lo:hi])

            scale_sec(0, 128)
            scale_sec(C, C + 128)

            def scale_rest():
                scale_sec(128, 256)
                scale_sec(C + 128, C + 256)
                scale_sec(2 * C, 3 * C)

            # ---- QKV projections ----
            # k2/q2: [128, pair, n] with head (2*pair + p//64) at partition
            # (p%64); produced directly by 128-wide output matmuls.
            k2 = big.tile([128, 2, HW], bf16, tag="k2")
            q2 = big.tile([128, 2, NQ], bf16, tag="q2")
            # vT8: [key_chunk_part, kc_pair, parity, h, 72] fp8e4m3; col 64 =
            # ones (denominator).  The (parity, h, 0:65) slice is the
            # DoubleRow lhsT [Ki=128, Ko=2, M=65]; dim padded 65->72 so the
            # Ko step (NH*72 = 288 B) is 16B-aligned as DoubleRow requires.
            vT8 = big.tile([128, 16, 2, NH, 72], f8, tag="vT8")
            onesc = const.tile([128, 1], f32, tag="onesc")
            nc.vector.memset(onesc, 1.0)
            nc.vector.tensor_copy(
                vT8[:, :, :, :, 64:65], onesc.to_broadcast((128, 16, 2, NH, 1))
            )

            def emit_q(pair, j4s):
                for j4 in j4s:
                    ps = psA_pool.tile([128, 512], f32, tag="psS")
                    nc.tensor.matmul(
                        ps,
                        lhsT=wt8[:, :, pair * 128 : (pair + 1) * 128],
                        rhs=x8[:, :, j4 * 512 : (j4 + 1) * 512],
                        start=True,
                        stop=True,
                        perf_mode=DR,
                    )
                    nc.vector.tensor_scalar_add(
                        q2[:, pair, j4 * 512 : (j4 + 1) * 512], ps,
                        qkvb2[:, pair : pair + 1],
                    )

            def emit_k(pair, n8s):
                # NOTE: the k bias is dropped entirely -- q.(Wk x + bk) =
                # q.Wk x + q.bk, and q.bk is constant across keys, so it
                # cancels in the softmax.  The PSUM->SBUF drain is a pure
                # copy and runs on ScalarE (Copy is in every activation
                # table set), rebalancing jp0's drain load off the DVE.
                for n8 in n8s:
                    ps = psA_pool.tile([128, 512], f32, tag="psS")
                    nc.tensor.matmul(
                        ps,
                        lhsT=wt8[:, :, C + pair * 128 : C + (pair + 1) * 128],
                        rhs=x8[:, :, n8 * 512 : (n8 + 1) * 512],
                        start=True,
                        stop=True,
                        perf_mode=DR,
                    )
                    nc.scalar.activation(
                        k2[:, pair, n8 * 512 : (n8 + 1) * 512], ps, AF.Copy
                    )

            def emit_vT(kcps):
                # one [128, 512] PSUM tile + ONE strided drain per chunk
                # PAIR (vs per chunk): halves the DVE drain instruction count
                for kcp in kcps:
                    ps = psA_pool.tile([128, 2, 256], f32, tag="psS")
                    for par in range(2):
                        nc.tensor.matmul(
                            ps[:, par, :],
                            lhsT=x8[:, :, (2 * kcp + par) * 128
                                    : (2 * kcp + par + 1) * 128],
                            rhs=wt8[:, :, 2 * C : 3 * C],
                            start=True,
                            stop=True,
                            perf_mode=DR,
                        )
                    nc.vector.tensor_copy(
                        vT8[:, kcp, :, :, 0:64],
                        ps.rearrange("p x (h d) -> p x h d", h=NH),
                    )

            # minimal prefix before attention starts: one k tile + one q tile
            # (chunks 0-3 only need k-tile 0).  Everything else -- remaining
            # weight-section folds, k tiles, vT chunks, pair-1 qkv -- streams
            # through jp0's deferred slots, paced just ahead of consumption,
            # so the first exp starts ~15us earlier.
            emit_k(0, [0, 1])
            emit_q(0, [0])
            # deferred emission schedule: {jp: {cyc: thunk}}; k tile n8 is
            # consumed from chunk 4*n8, emitted >=5 cycles earlier so its
            # drain never gates a score.
            deferred = {
                0: {
                    0: lambda: (scale_rest(), emit_vT(range(0, 2))),
                    1: lambda: (emit_k(0, [2]), emit_vT(range(2, 4))),
                    4: lambda: (emit_k(0, [3]), emit_vT(range(4, 6))),
                    7: lambda: (emit_k(0, [4]), emit_vT(range(6, 8))),
                    10: lambda: (emit_k(0, [5]), emit_vT(range(8, 10))),
                    13: lambda: (emit_k(0, [6]), emit_vT(range(10, 12))),
                    16: lambda: (emit_k(0, [7]), emit_vT(range(12, 14))),
                    19: lambda: emit_vT(range(14, 16)),
                    22: lambda: emit_k(1, range(0, 4)),
                    25: lambda: emit_k(1, range(4, 8)),
                    28: lambda: emit_q(1, [0]),
                },
                1: {3: lambda: emit_q(0, [1]), 13: lambda: emit_q(1, [1])},
                2: {3: lambda: emit_q(0, [2]), 13: lambda: emit_q(1, [2])},
                3: {3: lambda: emit_q(0, [3]), 13: lambda: emit_q(1, [3])},
            }

            def emit_xres(j):
                # residual fetch for query tile j, issued well before the
                # proj needs it so the DMA latency hides
                xres = work.tile([128, 2, 512], f32, tag="xres")
                nc.sync.dma_start(xres, x_r[:, :, j * 512 : (j + 1) * 512])
                return xres

            def emit_proj(j, att_j, xres):
                # proj + bias + residual for query tile j (emitted lazily so
                # the in-order PE stream never stalls on the division tail)
                y_sb = work.tile([128, 2, 512], f32, tag="y")
                for ot in range(2):
                    psY = psO_pool.tile([128, 512], f32, tag="psO")
                    for h in range(4):
                        nc.tensor.matmul(
                            psY,
                            lhsT=wproj_sb[:, h, ot * 128 : (ot + 1) * 128],
                            rhs=att_j[:, h, :],
                            start=(h == 0),
                            stop=(h == 3),
                        )
                    # y = (psY + projb2) + xres in one DVE op
                    nc.vector.scalar_tensor_tensor(
                        y_sb[:, ot, :], psY, projb2[:, ot : ot + 1],
                        xres[:, ot, :], ALU.add, ALU.add,
                    )
                nc.sync.dma_start(y_r[:, :, j * 512 : (j + 1) * 512], y_sb)

            pending = None
            for j in range(4):
                att_j = work.tile([64, NH, 512], bf16, tag="att")
                for pair in range(2):
                    jp = 2 * j + pair
                    dve_kcs = DVE_SCHED.get(jp, DVE_STEADY)
                    defer_jp = deferred.get(jp, {})
                    psO0 = psO_pool.tile([65, 512], f32, tag="psO")
                    psO1 = psO_pool.tile([65, 512], f32, tag="psO")
                    prev_st = None
                    live_P = {}
                    # lag-4 software pipeline on the in-order PE stream:
                    # cycle c emits scores(c) then PV(c-4).  The score tiles
                    # are freed by their exp (not the PV), so the PV lag is
                    # free to be deep -- by lag 4 the exp is always done and
                    # the in-order PE queue never stalls on a PV semaphore.
                    for cyc in range(36):
                        if cyc < 32:
                            kc = cyc
                            # scores for both heads of the pair in one tile:
                            # S[:, 0, :] head 2p (PE rows 0-63), S[:, 1, :]
                            # head 2p+1 (rows 64-127); the two matmuls are
                            # pc-adjacent so they overlap in the PE array.
                            S = (psD_pool if kc % 3 == 2 else psA_pool).tile(
                                [128, 2, 512], f32,
                                tag="psD" if kc % 3 == 2 else "psS",
                            )
                            ma = nc.tensor.matmul(
                                S[:, 0, :],
                                lhsT=k2[0:64, pair, kc * 128 : (kc + 1) * 128],
                                rhs=q2[0:64, pair, j * 512 : (j + 1) * 512],
                                start=True,
                                stop=True,
                            )
                            mb = nc.tensor.matmul(
                                S[:, 1, :],
                                lhsT=k2[64:128, pair, kc * 128 : (kc + 1) * 128],
                                rhs=q2[64:128, pair, j * 512 : (j + 1) * 512],
                                start=True,
                                stop=True,
                            )
                            if prev_st is not None:
                                add_dep_helper(
                                    ma.ins, prev_st, sync=False,
                                    reason="st-pair order",
                                )
                            add_dep_helper(
                                mb.ins, ma.ins, sync=False, reason="st-pair order"
                            )
                            prev_st = mb.ins
                            # exp writes e4m3 into the chunk-pair P tile:
                            # parity 0 (even kc, ScalarE exact exp w/ RNE
                            # cast) or parity 1 (odd kc, VectorE Schraudolph
                            # bits as uint8) -- the engines write disjoint
                            # slices concurrently.
                            if kc % 2 == 0:
                                Pp = pf8.tile([128, 2, 2, 512], f8, tag="Pp")
                                live_P[kc // 2] = Pp
                            else:
                                Pp = live_P[kc // 2]
                            par = kc % 2
                            if kc in dve_kcs:
                                nc.vector.tensor_scalar(
                                    Pp[:, par, :, :].bitcast(u8), S,
                                    A8, B8, ALU.mult, ALU.add,
                                )
                            else:
                                nc.scalar.activation(
                                    Pp[:, par, :, :], S, AF.Exp,
                                    bias=shift_sb, scale=float(HD) ** -0.5,
                                )
                        if cyc >= 5 and (cyc - 5) % 2 == 0:
                            # fp8 DoubleRow PV: one matmul per head covers a
                            # chunk PAIR (contraction 256 = 2 fp8 weights per
                            # PE cell, 0.5 cycles/column) -- halves the PE
                            # time of the PV side.
                            m = (cyc - 5) // 2
                            Pp = live_P.pop(m)
                            for hp, psO in ((0, psO0), (1, psO1)):
                                nc.tensor.matmul(
                                    psO,
                                    lhsT=vT8[:, m, :, 2 * pair + hp, 0:65],
                                    rhs=Pp[:, :, hp, :],
                                    start=(m == 0),
                                    stop=(m == 15),
                                    perf_mode=DR,
                                )
                        if cyc in defer_jp:
                            defer_jp[cyc]()
                        # emit_proj is deferred to cyc 14 of the next pair:
                        # the division chain for att_j (drain -> DMA spread /
                        # recip / broadcast -> gpsimd mul) takes ~10us after
                        # the last PV, and the proj matmuls sit in the
                        # in-order PE stream -- emitted too early they dam up
                        # everything behind them (scores AND the PV-DRs whose
                        # retirement the exp-engine P-tile rotation waits on,
                        # stalling the whole pair ~6.5us).  The xres fetch
                        # has no att dependency and issues at cyc 0.
                        if pending is not None and pair == 0:
                            if cyc == 0:
                                pending = pending + (emit_xres(pending[0]),)
                            elif cyc == 14:
                                emit_proj(*pending)
                                pending = None
                    # copy out of PSUM right away so the psO slots free for
                    # the next head pair; the division chain below works off
                    # the SBUF copy, off the critical path.  Both heads drain
                    # into one [65, 1024] SBUF tile so the reciprocal /
                    # broadcast chain runs once per pair instead of per head.
                    oc = work.tile([65, 1024], f32, tag="oc")
                    nc.scalar.activation(oc[:, 0:512], psO0, AF.Copy)
                    nc.scalar.activation(oc[:, 512:1024], psO1, AF.Copy)
                    # spread the denominator row over 64 partitions so the
                    # iterative DVE reciprocal (8 cyc/elem) is not
                    # single-lane-bound; the broadcast bounces through DRAM
                    # (SBUF DMA sources cannot have a zero partition step,
                    # and GpSimd-side alternatives measured slower).
                    r8 = work.tile([64, 16], f32, tag="r8")
                    nc.sync.dma_start(
                        r8, oc[64:65, :].rearrange("o (a b) -> o a b", b=16)
                    )
                    nc.vector.reciprocal(r8, r8)
                    rec_d = dram_pool.tile([1, 1024], f32, tag="recd")
                    nc.sync.dma_start(
                        rec_d.rearrange("o (a b) -> o a b", b=16), r8
                    )
                    rec_b = work.tile([64, 1024], f32, tag="recb")
                    nc.sync.dma_start(
                        rec_b, rec_d[0:1, :].to_broadcast((64, 1024))
                    )
                    for hp in (0, 1):
                        h = 2 * pair + hp
                        # attention scaling on GpSimd (otherwise idle)
                        nc.gpsimd.tensor_mul(
                            att_j[:, h, :],
                            oc[0:64, hp * 512 : (hp + 1) * 512],
                            rec_b[:, hp * 512 : (hp + 1) * 512],
                        )
                pending = (j, att_j)
            emit_proj(*pending, emit_xres(pending[0]))

    nc.compile()
    return nc


def _get_program():
    if "nc" not in _CACHE:
        _CACHE["nc"] = _build()
    return _CACHE["nc"]


def kernel(x, gn_w, gn_b, qkv_w, qkv_b, proj_w, proj_b):
    import ml_dtypes

    from concourse.bass_utils import run_bass_kernel_spmd

    x = np.asarray(x, np.float32)
    gn_w = np.asarray(gn_w, np.float32)
    gn_b = np.asarray(gn_b, np.float32)
    qkv_w = np.asarray(qkv_w, np.float32)
    qkv_b = np.asarray(qkv_b, np.float32)
    proj_w = np.asarray(proj_w, np.float32)
    proj_b = np.asarray(proj_b, np.float32)

    nc = _get_program()
    qkv_wt = np.ascontiguousarray(qkv_w.T).astype(ml_dtypes.bfloat16)
    proj_wt = np.ascontiguousarray(proj_w.T).astype(ml_dtypes.bfloat16)
    sel = np.zeros((128, 16), np.float32)
    sel[np.arange(128), np.arange(128) // 8] = 1.0
    selT = np.ascontiguousarray(sel.T)

    in_maps = []
    for core in range(8):
        b, half = core // 2, core % 2
        xb = x[b].reshape(C, HW)
        if half == 1:
            xb = np.concatenate([xb[:, NQ:], xb[:, :NQ]], axis=1)
        in_maps.append(
            {
                "x": np.ascontiguousarray(xb),
                "x_bf": np.ascontiguousarray(xb).astype(ml_dtypes.bfloat16),
                "x_f8": np.ascontiguousarray(xb).astype(ml_dtypes.float8_e4m3fn),
                "qkv_wt": qkv_wt,
                "qkv_b": qkv_b,
                "proj_wt": proj_wt,
                "proj_b": proj_b,
                "gn_w": gn_w,
                "gn_b": gn_b,
                "sel": sel,
                "selT": selT,
            }
        )

    res = run_bass_kernel_spmd(nc, in_maps, core_ids=list(range(8)))
    out = np.empty((B, C, HW), np.float32)
    for core in range(8):
        b, half = core // 2, core % 2
        out[b][:, half * NQ : (half + 1) * NQ] = res.results[core]["y"]
    return out.reshape(B, C, 64, 64)



# revision 59
# speedup vs baseline: 1.0094x; 1.0094x over previous
"""AttentionBlock kernel for 8 Trainium2 NeuronCores (v10, 255us HW).

Problem (hardcoded shapes): x [4, 256, 64, 64] f32.
  GroupNorm(32 groups) -> qkv 1x1 conv (768x256) -> 4-head attention over
  n=4096 tokens (hd=64) -> proj 1x1 conv -> residual add.

Sharding: 8 cores = (batch b in 0..3) x (query-half in 0..1).  Each core
computes GroupNorm + K/V for its whole batch image (duplicated across the
2 cores of a batch, cheap) and attention + proj + residual for its half of
the query positions (2048 of 4096).  Key order is permuted so the core's
query slice comes first; softmax is permutation-invariant over keys.

v10 structure (vs the 327us v4 baseline; all deltas HW-measured):
  - fp8e4m3 DoubleRow PV matmuls: each PV covers a chunk PAIR (contraction
    256 = 2 fp8 weights/PE cell, 0.5 cyc/col) -- halves the PV side of the
    PE stream, which paces the steady state.  P comes from ScalarE exact
    exp (even chunks, fp8 out) / VectorE uint8 Schraudolph (odd chunks);
    exp args are shifted by SHIFT8 (cancels in softmax) to center weights
    in e4m3 range.  V is cast to fp8 at drain time.
  - fp8 DoubleRow qkv projections too (x8/wt8 operands; GroupNorm stats
    still from the bf16 copy; GroupNorm affine folded into wt8 on-chip).
  - k-bias dropped entirely: q.(Wk x + bk) adds a per-query constant to
    scores, which softmax cancels.  k2 drains are pure ScalarE copies,
    balancing jp0's drain load across both act engines.
  - HAM management: the PE clock gate starts throttled (1.2 vs 2.4 GHz)
    and re-throttles after idle windows.  Dummy matmuls (free-running +
    bn_stats-paced) keep the PE busy through the GroupNorm prologue; the
    steady stream then holds one continuous ~200us warm window.
  - GroupNorm rstd via fast-inverse-sqrt + 1 Newton step on the DVE: the
    Exp activation table is the only set ScalarE ever loads (a Ln<->Exp
    switch costs 1.3us and thrashed per-t when scheduled naively).
  - emit_proj deferred to cyc 14 of the NEXT pair: the att division chain
    (drain -> DMA spread -> recip -> DRAM-bounce broadcast -> GpSimd mul)
    is ~10us of DMA latency; proj matmuls emitted earlier dam up the
    in-order PE stream (and the P-tile pool rotation the exps wait on),
    which cost ~6.5us + a HAM re-throttle per j boundary.
  - Softmax exp split ~16/16 between ScalarE (exact) and VectorE
    (Schraudolph); both land ~88% busy, just under the PE pace.
"""

import sys

import numpy as np

sys.path.insert(0, "/opt/trn_rl_repo")

B, C, HW = 4, 256, 4096
NQ = HW // 2  # queries per core
NH, HD = 4, 64
G = 32  # groups
EPS = 1e-5

# Per-(j,pair) exp split: which key chunks (0..31) VectorE handles.
# jp = 2*j + pair. Early jps keep VectorE light (it also drains the qkv
# projections); never the last chunks (they gate the psO drain).
# NOTE: pool placement (psA vs psD) is fixed by kc % 3 == 2 regardless of
# engine -- the lag-2 PE pipeline keeps 3 score tiles live, which exactly
# fits psA bufs=2 + psD bufs=1 when psD takes every third chunk.
# Engine pattern alternates so neither engine ever gets two consecutive
# chunks (a 2:1 run structure makes the faster engine the 765ns/chunk
# pacer); chunk 31 stays on ScalarE so the psO drain is not queued
# behind VectorE exps.
DVE_SCHED = {
    0: (1, 3, 7, 9, 13, 15, 19, 21, 25, 27, 29),
    1: (1, 3, 5, 7, 9, 11, 13, 15, 19, 21, 23, 25, 27, 29),
}
DVE_STEADY = (0, 2, 4, 6, 8, 10, 12, 14, 16, 18, 20, 22, 24, 26, 28, 30)

# Schraudolph constants, fp8e4m3 flavor: byte = trunc(x_raw * A8 + B8)
# approximates exp(x_raw/8 - SHIFT8) as e4m3 bits.  The softmax is
# shift-invariant, so SHIFT8 cancels exactly (the ones-column denominator
# is scaled identically); it just centers the weights in e4m3's range so
# almost nothing is subnormal or clamped.  Numerically validated: max
# softmax-output error ~5e-3 (vs out scale 0.1) with fp8 V, ~2.4x the old
# bf16 trick, still ~10x inside the rel-err gate.
LOG2E = 1.4426950408889634
SHIFT8 = 2.0
A8 = LOG2E
B8 = 56.0 - 8.0 * SHIFT8 * LOG2E

# HAM warmup dummy-matmul counts (see _build): free-running batch from
# t~6.5us, then a few per bn_stats completion to pace through the prologue.
N_WARM_FREE = 14
N_WARM_PER_STAT = 3

_CACHE = {}


def _build():
    import concourse.bass as bass
    import concourse.tile as tile
    from concourse import bacc, mybir
    from concourse.tile import add_dep_helper

    f32 = mybir.dt.float32
    bf16 = mybir.dt.bfloat16
    u8 = mybir.dt.uint8
    f8 = mybir.dt.float8e4
    AF = mybir.ActivationFunctionType
    ALU = mybir.AluOpType
    DR = mybir.MatmulPerfMode.DoubleRow

    nc = bacc.Bacc(
        "TRN2",
        target_bir_lowering=False,
        debug=False,
        enable_asserts=False,
        num_devices=8,
    )

    x_d = nc.dram_tensor("x", [C, HW], f32, kind="ExternalInput").ap()
    x_bf_d = nc.dram_tensor("x_bf", [C, HW], bf16, kind="ExternalInput").ap()
    x_f8_d = nc.dram_tensor("x_f8", [C, HW], f8, kind="ExternalInput").ap()
    qkv_wt_d = nc.dram_tensor("qkv_wt", [C, 3 * C], bf16, kind="ExternalInput").ap()
    qkv_b_d = nc.dram_tensor("qkv_b", [3 * C], f32, kind="ExternalInput").ap()
    proj_wt_d = nc.dram_tensor("proj_wt", [C, C], bf16, kind="ExternalInput").ap()
    proj_b_d = nc.dram_tensor("proj_b", [C], f32, kind="ExternalInput").ap()
    gn_w_d = nc.dram_tensor("gn_w", [C], f32, kind="ExternalInput").ap()
    gn_b_d = nc.dram_tensor("gn_b", [C], f32, kind="ExternalInput").ap()
    sel_d = nc.dram_tensor("sel", [128, 16], f32, kind="ExternalInput").ap()
    selT_d = nc.dram_tensor("selT", [16, 128], f32, kind="ExternalInput").ap()
    y_d = nc.dram_tensor("y", [C, NQ], f32, kind="ExternalOutput").ap()

    x_r = x_d.rearrange("(t p) n -> p t n", p=128)  # c = t*128 + p
    x_bf_r = x_bf_d.rearrange("(t p) n -> p t n", p=128)
    x_f8_r = x_f8_d.rearrange("(t p) n -> p t n", p=128)
    y_r = y_d.rearrange("(t p) n -> p t n", p=128)

    with tile.TileContext(nc) as tc:
        with (
            tc.tile_pool(name="const", bufs=1) as const,
            tc.tile_pool(name="big", bufs=1) as big,
            tc.tile_pool(name="work", bufs=2) as work,
            tc.tile_pool(name="pf8", bufs=6) as pf8,
            tc.tile_pool(name="psA", bufs=2, space="PSUM") as psA_pool,
            tc.tile_pool(name="psD", bufs=1, space="PSUM") as psD_pool,
            tc.tile_pool(name="psO", bufs=2, space="PSUM") as psO_pool,
            tc.tile_pool(name="dram", bufs=2, space="DRAM") as dram_pool,
        ):
            # ---- x load FIRST: everything gates on GroupNorm stats over the
            # full image, and each DMA costs ~600ns of issue time on the Sync
            # queue -- so the x transfers go ahead of all const DMAs.  Split
            # into 8 chunks so bn_stats can start on the first quarter while
            # the rest streams in.
            xb = big.tile([128, 2, HW], bf16, tag="xb")
            for t in range(2):
                for q4 in range(4):
                    nc.sync.dma_start(
                        xb[:, t, q4 * 1024 : (q4 + 1) * 1024],
                        x_bf_r[:, t, q4 * 1024 : (q4 + 1) * 1024],
                    )
            # fp8 copy of x for the qkv DoubleRow matmuls (bf16 copy above
            # still feeds the GroupNorm stats); needed from ~24us on.
            x8 = big.tile([128, 2, HW], f8, tag="x8")
            for t in range(2):
                nc.sync.dma_start(x8[:, t, :], x_f8_r[:, t, :])

            # ---- HAM warmup + activation-table preload ----
            # The PE clock gate (HAM) only un-throttles (1.2 -> 2.4 GHz)
            # after ~3.4us of sustained matmul activity, and re-throttles
            # after an idle window.  The GroupNorm-stats prologue leaves the
            # PE idle for ~15us, so the whole first attention stream used to
            # run at half clock.  Dummy matmuls (results never read) keep the
            # PE busy from t~6.5us until the real stream starts: a free-run
            # batch first, then batches paced by bn_stats completions so they
            # stretch to the end of the stats chain without delaying it.
            warm = const.tile([128, 512], bf16, tag="warm")
            nc.vector.memset(warm, 0.25)

            def dummy_mm(dep=None):
                # shares the psO tag so the pool stays at 2 PSUM banks; the
                # warmup rotation retires long before the first real psO.
                psW = psO_pool.tile([128, 512], f32, tag="psO")
                m = nc.tensor.matmul(
                    psW, lhsT=warm[:, 0:128], rhs=warm, start=True, stop=True
                )
                if dep is not None:
                    add_dep_helper(m.ins, dep.ins, sync=True, reason="warm pace")

            # The Exp activation table loads on first use (~1.3us); trigger
            # the load now while ScalarE is idle instead of on the GroupNorm
            # critical path.  Exp is the ONLY table set ever used (rstd is
            # computed on the DVE below), so there is no table switching.
            tpre = const.tile([1, 8], f32, tag="tpre")
            nc.vector.memset(tpre, 1.0)
            nc.scalar.activation(tpre, tpre, AF.Exp)
            # per-partition bias tile for the fp8 exp shift
            shift_sb = const.tile([128, 1], f32, tag="shift8")
            nc.vector.memset(shift_sb, -SHIFT8)

            for _ in range(N_WARM_FREE):
                dummy_mm()

            # ---- constants / weights ----
            wt_sb = const.tile([128, 2, 3 * C], bf16, tag="wt")
            nc.sync.dma_start(wt_sb, qkv_wt_d.rearrange("(t p) o -> p t o", p=128))
            wproj_sb = const.tile([64, NH, C], bf16, tag="wproj")
            nc.sync.dma_start(wproj_sb, proj_wt_d.rearrange("(h p) o -> p h o", p=64))
            qkvb_sb = const.tile([128, 6], f32, tag="qkvb")
            nc.sync.dma_start(qkvb_sb, qkv_b_d.rearrange("(s p) -> p s", p=128))
            vb_sb = const.tile([64, NH], f32, tag="vb")
            nc.sync.dma_start(vb_sb, qkv_b_d[2 * C :].rearrange("(h p) -> p h", p=64))
            projb_sb = const.tile([128, 2], f32, tag="projb")
            nc.sync.dma_start(projb_sb, proj_b_d.rearrange("(t p) -> p t", p=128))
            gnw_sb = const.tile([128, 2], f32, tag="gnw")
            nc.sync.dma_start(gnw_sb, gn_w_d.rearrange("(t p) -> p t", p=128))
            gnb_sb = const.tile([128, 2], f32, tag="gnb")
            nc.sync.dma_start(gnb_sb, gn_b_d.rearrange("(t p) -> p t", p=128))

            # group-selector matrices (channels<->groups), used for the tiny
            # cross-partition reductions in GroupNorm stats.
            ones1 = const.tile([1, 64], f32, tag="ones1")
            nc.vector.memset(ones1, 1.0)
            sel = const.tile([128, 16], f32, tag="sel")  # sel[p, g]=1 if p//8==g
            nc.sync.dma_start(sel, sel_d)
            selT = const.tile([16, 128], f32, tag="selT")
            nc.sync.dma_start(selT, selT_d)

            # ---- GroupNorm stats (bn_stats free dim is capped at 512) ----
            stats = const.tile([128, 2, 8, 6], f32, tag="stats")
            mv = const.tile([128, 2, 2], f32, tag="mv")
            st_handles = []
            for t in range(2):
                for jj in range(8):
                    st = nc.vector.bn_stats(
                        stats[:, t, jj, :], xb[:, t, jj * 512 : (jj + 1) * 512]
                    )
                    st_handles.append(st)
                nc.vector.bn_aggr(mv[:, t, :], stats[:, t])
            # paced warmup: a few dummy matmuls gated on each bn_stats so the
            # PE never idles past the HAM re-throttle window while the DVE
            # stats chain (the prologue critical path) runs.
            for st in st_handles:
                for _ in range(N_WARM_PER_STAT):
                    dummy_mm(dep=st)
            # me2[:, t, 0] = mean_c ; me2[:, t, 1] = E[x^2]_c = var + mean^2
            me2 = const.tile([128, 2, 2], f32, tag="me2")
            for t in range(2):
                nc.vector.tensor_copy(me2[:, t, 0:1], mv[:, t, 0:1])
                nc.vector.tensor_mul(me2[:, t, 1:2], mv[:, t, 0:1], mv[:, t, 0:1])
                nc.vector.tensor_add(me2[:, t, 1:2], me2[:, t, 1:2], mv[:, t, 1:2])
            # group sums via selector matmul (fp32 for exactness)
            psg = psA_pool.tile([16, 2, 2], f32, tag="psS")
            for t in range(2):
                nc.tensor.matmul(
                    psg[:, t, :], lhsT=sel, rhs=me2[:, t, :], start=True, stop=True
                )
            gsb = const.tile([16, 2, 2], f32, tag="gsb")
            gmr = const.tile([16, 2, 2], f32, tag="gmr")  # (mean_g, rstd_g)
            # rstd = rsqrt(var+eps) on the DVE via fast-inverse-sqrt + one
            # Newton step (rel err ~2e-3, noise vs the ~3% Schraudolph
            # ripple).  Keeping this off ScalarE means the Exp activation
            # table is the only set ever loaded -- the Ln<->Exp table
            # switches (1.3us each) this replaces sat on the critical path
            # and could thrash per-t.
            i32 = mybir.dt.int32
            magic = const.tile([16, 1], i32, tag="magic")
            nc.vector.memset(magic, 0x5F3759DF)
            rsq = const.tile([16, 2, 8], f32, tag="rsq")
            for t in range(2):
                nc.vector.tensor_scalar_mul(gsb[:, t, :], psg[:, t, :], 1.0 / 8.0)
                nc.vector.tensor_copy(gmr[:, t, 0:1], gsb[:, t, 0:1])
                # var_g = E2_g - mean_g^2
                nc.vector.tensor_mul(gmr[:, t, 1:2], gsb[:, t, 0:1], gsb[:, t, 0:1])
                nc.vector.tensor_sub(gmr[:, t, 1:2], gsb[:, t, 1:2], gmr[:, t, 1:2])
                ve = rsq[:, t, 0:1]  # var + eps
                nc.vector.tensor_scalar(
                    ve, gmr[:, t, 1:2], EPS, 0.0, ALU.add, ALU.bypass
                )
                # y0 bits = magic - (bits(ve) >> 1)
                sh = rsq[:, t, 1:2]
                nc.vector.tensor_scalar(
                    sh.bitcast(i32), ve.bitcast(i32), 1, 0,
                    ALU.logical_shift_right, ALU.bypass,
                )
                y0 = rsq[:, t, 2:3]
                nc.vector.tensor_sub(y0.bitcast(i32), magic, sh.bitcast(i32))
                # one Newton step: y1 = y0 * (1.5 - 0.5*ve*y0^2)
                aa = rsq[:, t, 3:4]
                nc.vector.tensor_mul(aa, y0, y0)
                nc.vector.tensor_mul(aa, aa, ve)
                nc.vector.tensor_scalar(aa, aa, -0.5, 1.5, ALU.mult, ALU.add)
                nc.vector.tensor_mul(gmr[:, t, 1:2], y0, aa)
            # broadcast group stats back to channels
            psb = psA_pool.tile([128, 2, 2], f32, tag="psS")
            for t in range(2):
                nc.tensor.matmul(
                    psb[:, t, :], lhsT=selT, rhs=gmr[:, t, :], start=True, stop=True
                )
            # per-channel affine: xn = x * a + bcoef  (xn in bf16 for PE)
            ab = const.tile([128, 2, 2], f32, tag="ab")
            for t in range(2):
                nc.vector.tensor_mul(ab[:, t, 0:1], psb[:, t, 1:2], gnw_sb[:, t : t + 1])
                nc.vector.tensor_mul(ab[:, t, 1:2], psb[:, t, 0:1], ab[:, t, 0:1])
                nc.vector.tensor_sub(ab[:, t, 1:2], gnb_sb[:, t : t + 1], ab[:, t, 1:2])
            # Fold the GroupNorm affine into the qkv weights instead of
            # normalizing x:  W(ax+b) + c = (Wa)x + (Wb + c).
            bvec = const.tile([128, 2, 1], bf16, tag="bvec")
            for t in range(2):
                nc.vector.tensor_copy(bvec[:, t, :], ab[:, t, 1:2])
            qkvb2 = const.tile([128, 6], f32, tag="qkvb2")
            for s in range(2):  # q 128-wide output sections (k bias cancels)
                psq = psA_pool.tile([128, 1], f32, tag="psS")
                for t in range(2):
                    nc.tensor.matmul(
                        psq,
                        lhsT=wt_sb[:, t, s * 128 : (s + 1) * 128],
                        rhs=bvec[:, t, :],
                        start=(t == 0),
                        stop=(t == 1),
                    )
                nc.vector.tensor_add(qkvb2[:, s : s + 1], qkvb_sb[:, s : s + 1], psq)
            vb2 = const.tile([64, NH], f32, tag="vb2")
            for h in range(NH):  # v bias per head (64-wide, base partition 0)
                psv = psA_pool.tile([64, 1], f32, tag="psS")
                for t in range(2):
                    nc.tensor.matmul(
                        psv,
                        lhsT=wt_sb[:, t, 2 * C + h * 64 : 2 * C + (h + 1) * 64],
                        rhs=bvec[:, t, :],
                        start=(t == 0),
                        stop=(t == 1),
                    )
                nc.vector.tensor_add(vb2[:, h : h + 1], vb_sb[:, h : h + 1], psv)
            # out = PV/denom + vb2 exactly, so the v bias folds into the proj
            # bias: projb2 = proj_b + proj_w @ vb2.
            vb2bf = const.tile([64, NH], bf16, tag="vb2bf")
            nc.vector.tensor_copy(vb2bf, vb2)
            projb2 = const.tile([128, 2], f32, tag="projb2")
            for ot in range(2):
                psB = psA_pool.tile([128, 1], f32, tag="psS")
                for h in range(NH):
                    nc.tensor.matmul(
                        psB,
                        lhsT=wproj_sb[:, h, ot * 128 : (ot + 1) * 128],
                        rhs=vb2bf[:, h : h + 1],
                        start=(h == 0),
                        stop=(h == 3),
                    )
                nc.vector.tensor_add(projb2[:, ot : ot + 1], projb_sb[:, ot : ot + 1], psB)
            # scale the weight columns in place (after the bias matmuls),
            # then cast the scaled section to fp8 for the DoubleRow qkv
            # matmuls (contraction 256 = both t-halves in one matmul at 0.5
            # cycles/column).  Only the pair-0 q/k sections are done up
            # front -- they gate the first score chunk; the rest is deferred
            # into the jp0 stream where the first exps provide cover.
            wt8 = const.tile([128, 2, 3 * C], f8, tag="wt8")

            def scale_sec(lo, hi):
                for t in range(2):
                    nc.vector.tensor_scalar_mul(
                        wt_sb[:, t, lo:hi], wt_sb[:, t, lo:hi], ab[:, t, 0:1]
                    )
                nc.vector.tensor_copy(wt8[:, :, lo:hi], wt_sb[:, :, lo:hi])

            scale_sec(0, 128)
            scale_sec(C, C + 128)
            scale_sec(2 * C, 3 * C)  # v section early: gates the vT prefix

            def scale_rest():
                scale_sec(128, 256)
                scale_sec(C + 128, C + 256)

            # ---- QKV projections ----
            # k2/q2: [128, pair, n] with head (2*pair + p//64) at partition
            # (p%64); produced directly by 128-wide output matmuls.
            k2 = big.tile([128, 2, HW], bf16, tag="k2")
            q2 = big.tile([128, 2, NQ], bf16, tag="q2")
            # vT8: [key_chunk_part, kc_pair, parity, h, 72] fp8e4m3; col 64 =
            # ones (denominator).  The (parity, h, 0:65) slice is the
            # DoubleRow lhsT [Ki=128, Ko=2, M=65]; dim padded 65->72 so the
            # Ko step (NH*72 = 288 B) is 16B-aligned as DoubleRow requires.
            vT8 = big.tile([128, 16, 2, NH, 72], f8, tag="vT8")
            onesc = const.tile([128, 1], f32, tag="onesc")
            nc.vector.memset(onesc, 1.0)
            nc.vector.tensor_copy(
                vT8[:, :, :, :, 64:65], onesc.to_broadcast((128, 16, 2, NH, 1))
            )

            def emit_q(pair, j4s):
                for j4 in j4s:
                    ps = psA_pool.tile([128, 512], f32, tag="psS")
                    nc.tensor.matmul(
                        ps,
                        lhsT=wt8[:, :, pair * 128 : (pair + 1) * 128],
                        rhs=x8[:, :, j4 * 512 : (j4 + 1) * 512],
                        start=True,
                        stop=True,
                        perf_mode=DR,
                    )
                    nc.vector.tensor_scalar_add(
                        q2[:, pair, j4 * 512 : (j4 + 1) * 512], ps,
                        qkvb2[:, pair : pair + 1],
                    )

            def emit_k(pair, n8s):
                # NOTE: the k bias is dropped entirely -- q.(Wk x + bk) =
                # q.Wk x + q.bk, and q.bk is constant across keys, so it
                # cancels in the softmax.  The PSUM->SBUF drain is a pure
                # copy and runs on ScalarE (Copy is in every activation
                # table set), rebalancing jp0's drain load off the DVE.
                for n8 in n8s:
                    ps = psA_pool.tile([128, 512], f32, tag="psS")
                    nc.tensor.matmul(
                        ps,
                        lhsT=wt8[:, :, C + pair * 128 : C + (pair + 1) * 128],
                        rhs=x8[:, :, n8 * 512 : (n8 + 1) * 512],
                        start=True,
                        stop=True,
                        perf_mode=DR,
                    )
                    nc.scalar.activation(
                        k2[:, pair, n8 * 512 : (n8 + 1) * 512], ps, AF.Copy
                    )

            def emit_vT(kcps):
                # one [128, 512] PSUM tile + ONE strided drain per chunk
                # PAIR (vs per chunk): halves the DVE drain instruction count
                for kcp in kcps:
                    ps = psA_pool.tile([128, 2, 256], f32, tag="psS")
                    for par in range(2):
                        nc.tensor.matmul(
                            ps[:, par, :],
                            lhsT=x8[:, :, (2 * kcp + par) * 128
                                    : (2 * kcp + par + 1) * 128],
                            rhs=wt8[:, :, 2 * C : 3 * C],
                            start=True,
                            stop=True,
                            perf_mode=DR,
                        )
                    nc.vector.tensor_copy(
                        vT8[:, kcp, :, :, 0:64],
                        ps.rearrange("p x (h d) -> p x h d", h=NH),
                    )

            # minimal prefix before attention starts: one k tile + one q tile
            # (chunks 0-3 only need k-tile 0).  Everything else -- remaining
            # weight-section folds, k tiles, vT chunks, pair-1 qkv -- streams
            # through jp0's deferred slots, paced just ahead of consumption,
            # so the first exp starts ~15us earlier.
            emit_k(0, [0, 1])
            emit_q(0, [0])
            # prefix continues with real work gated on the weight casts: it
            # lands exactly in the post-stats window where the PE otherwise
            # idles long enough to trip a HAM re-throttle (the first ~15us
            # of jp0 then ran at half clock), and it thins jp0's deferred
            # stream.
            emit_k(0, [2, 3])
            emit_vT(range(0, 4))
            # deferred emission schedule: {jp: {cyc: thunk}}; k tile n8 is
            # consumed from chunk 4*n8, emitted >=5 cycles earlier so its
            # drain never gates a score.
            deferred = {
                0: {
                    0: lambda: (scale_rest(), emit_vT(range(4, 6))),
                    1: lambda: (emit_k(0, [4]), emit_vT(range(6, 8))),
                    4: lambda: (emit_k(0, [5]), emit_vT(range(8, 10))),
                    7: lambda: (emit_k(0, [6]), emit_vT(range(10, 12))),
                    10: lambda: (emit_k(0, [7]), emit_vT(range(12, 14))),
                    13: lambda: emit_vT(range(14, 16)),
                    22: lambda: emit_k(1, range(0, 4)),
                    25: lambda: emit_k(1, range(4, 8)),
                    28: lambda: emit_q(1, [0]),
                },
                1: {3: lambda: emit_q(0, [1]), 13: lambda: emit_q(1, [1])},
                2: {3: lambda: emit_q(0, [2]), 13: lambda: emit_q(1, [2])},
                3: {3: lambda: emit_q(0, [3]), 13: lambda: emit_q(1, [3])},
            }

            def emit_xres(j):
                # residual fetch for query tile j, issued well before the
                # proj needs it so the DMA latency hides
                xres = work.tile([128, 2, 512], f32, tag="xres")
                nc.sync.dma_start(xres, x_r[:, :, j * 512 : (j + 1) * 512])
                return xres

            def emit_proj(j, att_j, xres):
                # proj + bias + residual for query tile j (emitted lazily so
                # the in-order PE stream never stalls on the division tail)
                y_sb = work.tile([128, 2, 512], f32, tag="y")
                for ot in range(2):
                    psY = psO_pool.tile([128, 512], f32, tag="psO")
                    for h in range(4):
                        nc.tensor.matmul(
                            psY,
                            lhsT=wproj_sb[:, h, ot * 128 : (ot + 1) * 128],
                            rhs=att_j[:, h, :],
                            start=(h == 0),
                            stop=(h == 3),
                        )
                    # y = (psY + projb2) + xres in one DVE op
                    nc.vector.scalar_tensor_tensor(
                        y_sb[:, ot, :], psY, projb2[:, ot : ot + 1],
                        xres[:, ot, :], ALU.add, ALU.add,
                    )
                nc.sync.dma_start(y_r[:, :, j * 512 : (j + 1) * 512], y_sb)

            pending = None
            for j in range(4):
                att_j = work.tile([64, NH, 512], bf16, tag="att")
                for pair in range(2):
                    jp = 2 * j + pair
                    dve_kcs = DVE_SCHED.get(jp, DVE_STEADY)
                    defer_jp = deferred.get(jp, {})
                    psO0 = psO_pool.tile([65, 512], f32, tag="psO")
                    psO1 = psO_pool.tile([65, 512], f32, tag="psO")
                    prev_st = None
                    live_P = {}
                    # lag-4 software pipeline on the in-order PE stream:
                    # cycle c emits scores(c) then PV(c-4).  The score tiles
                    # are freed by their exp (not the PV), so the PV lag is
                    # free to be deep -- by lag 4 the exp is always done and
                    # the in-order PE queue never stalls on a PV semaphore.
                    for cyc in range(36):
                        if cyc < 32:
                            kc = cyc
                            # scores for both heads of the pair in one tile:
                            # S[:, 0, :] head 2p (PE rows 0-63), S[:, 1, :]
                            # head 2p+1 (rows 64-127); the two matmuls are
                            # pc-adjacent so they overlap in the PE array.
                            S = (psD_pool if kc % 3 == 2 else psA_pool).tile(
                                [128, 2, 512], f32,
                                tag="psD" if kc % 3 == 2 else "psS",
                            )
                            ma = nc.tensor.matmul(
                                S[:, 0, :],
                                lhsT=k2[0:64, pair, kc * 128 : (kc + 1) * 128],
                                rhs=q2[0:64, pair, j * 512 : (j + 1) * 512],
                                start=True,
                                stop=True,
                            )
                            mb = nc.tensor.matmul(
                                S[:, 1, :],
                                lhsT=k2[64:128, pair, kc * 128 : (kc + 1) * 128],
                                rhs=q2[64:128, pair, j * 512 : (j + 1) * 512],
                                start=True,
                                stop=True,
                            )
                            if prev_st is not None:
                                add_dep_helper(
                                    ma.ins, prev_st, sync=False,
                                    reason="st-pair order",
                                )
                            add_dep_helper(
                                mb.ins, ma.ins, sync=False, reason="st-pair order"
                            )
                            prev_st = mb.ins
                            # exp writes e4m3 into the chunk-pair P tile:
                            # parity 0 (even kc, ScalarE exact exp w/ RNE
                            # cast) or parity 1 (odd kc, VectorE Schraudolph
                            # bits as uint8) -- the engines write disjoint
                            # slices concurrently.
                            if kc % 2 == 0:
                                Pp = pf8.tile([128, 2, 2, 512], f8, tag="Pp")
                                live_P[kc // 2] = Pp
                            else:
                                Pp = live_P[kc // 2]
                            par = kc % 2
                            if kc in dve_kcs:
                                nc.vector.tensor_scalar(
                                    Pp[:, par, :, :].bitcast(u8), S,
                                    A8, B8, ALU.mult, ALU.add,
                                )
                            else:
                                nc.scalar.activation(
                                    Pp[:, par, :, :], S, AF.Exp,
                                    bias=shift_sb, scale=float(HD) ** -0.5,
                                )
                        if cyc >= 5 and (cyc - 5) % 2 == 0:
                            # fp8 DoubleRow PV: one matmul per head covers a
                            # chunk PAIR (contraction 256 = 2 fp8 weights per
                            # PE cell, 0.5 cycles/column) -- halves the PE
                            # time of the PV side.
                            m = (cyc - 5) // 2
                            Pp = live_P.pop(m)
                            for hp, psO in ((0, psO0), (1, psO1)):
                                nc.tensor.matmul(
                                    psO,
                                    lhsT=vT8[:, m, :, 2 * pair + hp, 0:65],
                                    rhs=Pp[:, :, hp, :],
                                    start=(m == 0),
                                    stop=(m == 15),
                                    perf_mode=DR,
                                )
                        if cyc in defer_jp:
                            defer_jp[cyc]()
                        # emit_proj is deferred to cyc 14 of the next pair:
                        # the division chain for att_j (drain -> DMA spread /
                        # recip / broadcast -> gpsimd mul) takes ~10us after
                        # the last PV, and the proj matmuls sit in the
                        # in-order PE stream -- emitted too early they dam up
                        # everything behind them (scores AND the PV-DRs whose
                        # retirement the exp-engine P-tile rotation waits on,
                        # stalling the whole pair ~6.5us).  The xres fetch
                        # has no att dependency and issues at cyc 0.
                        if pending is not None and pair == 0:
                            if cyc == 0:
                                pending = pending + (emit_xres(pending[0]),)
                            elif cyc == 14:
                                emit_proj(*pending)
                                pending = None
                    # copy out of PSUM right away so the psO slots free for
                    # the next head pair; the division chain below works off
                    # the SBUF copy, off the critical path.  Both heads drain
                    # into one [65, 1024] SBUF tile so the reciprocal /
                    # broadcast chain runs once per pair instead of per head.
                    oc = work.tile([65, 1024], f32, tag="oc")
                    nc.scalar.activation(oc[:, 0:512], psO0, AF.Copy)
                    nc.scalar.activation(oc[:, 512:1024], psO1, AF.Copy)
                    # spread the denominator row over 64 partitions so the
                    # iterative DVE reciprocal (8 cyc/elem) is not
                    # single-lane-bound; the broadcast bounces through DRAM
                    # (SBUF DMA sources cannot have a zero partition step,
                    # and GpSimd-side alternatives measured slower).
                    r8 = work.tile([64, 16], f32, tag="r8")
                    nc.sync.dma_start(
                        r8, oc[64:65, :].rearrange("o (a b) -> o a b", b=16)
                    )
                    nc.vector.reciprocal(r8, r8)
                    rec_d = dram_pool.tile([1, 1024], f32, tag="recd")
                    nc.sync.dma_start(
                        rec_d.rearrange("o (a b) -> o a b", b=16), r8
                    )
                    rec_b = work.tile([64, 1024], f32, tag="recb")
                    nc.sync.dma_start(
                        rec_b, rec_d[0:1, :].to_broadcast((64, 1024))
                    )
                    for hp in (0, 1):
                        h = 2 * pair + hp
                        # attention scaling on GpSimd (otherwise idle)
                        nc.gpsimd.tensor_mul(
                            att_j[:, h, :],
                            oc[0:64, hp * 512 : (hp + 1) * 512],
                            rec_b[:, hp * 512 : (hp + 1) * 512],
                        )
                pending = (j, att_j)
            emit_proj(*pending, emit_xres(pending[0]))

    nc.compile()
    return nc


def _get_program():
    if "nc" not in _CACHE:
        _CACHE["nc"] = _build()
    return _CACHE["nc"]


def kernel(x, gn_w, gn_b, qkv_w, qkv_b, proj_w, proj_b):
    import ml_dtypes

    from concourse.bass_utils import run_bass_kernel_spmd

    x = np.asarray(x, np.float32)
    gn_w = np.asarray(gn_w, np.float32)
    gn_b = np.asarray(gn_b, np.float32)
    qkv_w = np.asarray(qkv_w, np.float32)
    qkv_b = np.asarray(qkv_b, np.float32)
    proj_w = np.asarray(proj_w, np.float32)
    proj_b = np.asarray(proj_b, np.float32)

    nc = _get_program()
    qkv_wt = np.ascontiguousarray(qkv_w.T).astype(ml_dtypes.bfloat16)
    proj_wt = np.ascontiguousarray(proj_w.T).astype(ml_dtypes.bfloat16)
    sel = np.zeros((128, 16), np.float32)
    sel[np.arange(128), np.arange(128) // 8] = 1.0
    selT = np.ascontiguousarray(sel.T)

    in_maps = []
    for core in range(8):
        b, half = core // 2, core % 2
        xb = x[b].reshape(C, HW)
        if half == 1:
            xb = np.concatenate([xb[:, NQ:], xb[:, :NQ]], axis=1)
        in_maps.append(
            {
                "x": np.ascontiguousarray(xb),
                "x_bf": np.ascontiguousarray(xb).astype(ml_dtypes.bfloat16),
                "x_f8": np.ascontiguousarray(xb).astype(ml_dtypes.float8_e4m3fn),
                "qkv_wt": qkv_wt,
                "qkv_b": qkv_b,
                "proj_wt": proj_wt,
                "proj_b": proj_b,
                "gn_w": gn_w,
                "gn_b": gn_b,
                "sel": sel,
                "selT": selT,
            }
        )

    res = run_bass_kernel_spmd(nc, in_maps, core_ids=list(range(8)))
    out = np.empty((B, C, HW), np.float32)
    for core in range(8):
        b, half = core // 2, core % 2
        out[b][:, half * NQ : (half + 1) * NQ] = res.results[core]["y"]
    return out.reshape(B, C, 64, 64)



# revision 60
# speedup vs baseline: 1.0101x; 1.0007x over previous
"""AttentionBlock kernel for 8 Trainium2 NeuronCores (v4).

Problem (hardcoded shapes): x [4, 256, 64, 64] f32.
  GroupNorm(32 groups) -> qkv 1x1 conv (768x256) -> 4-head attention over
  n=4096 tokens (hd=64) -> proj 1x1 conv -> residual add.

Sharding: 8 cores = (batch b in 0..3) x (query-half in 0..1).  Each core
computes GroupNorm + K/V for its whole batch image (duplicated across the
2 cores of a batch, cheap) and attention + proj + residual for its half of
the query positions (2048 of 4096).  Key order is permuted so the core's
query slice comes first; softmax is permutation-invariant over keys.

v4 structure (vs the 366us exp-bound baseline):
  - Softmax exp is split between ScalarE (exact exp) and VectorE
    (Schraudolph bit-trick: one tensor_scalar mult+add producing int16
    bf16-bit-patterns, bitcast to bf16 for the PV matmul; +-3% per-weight
    ripple that washes out in the softmax average). Split is per key
    chunk, ~12/32 chunks to VectorE in steady state, fewer early while
    VectorE also drains the qkv projections.
  - Both heads of the pair share one [128, 2(head), 512] score tile, so
    each chunk needs ONE exp instruction (N=1024) on its engine.
  - Scores for ScalarE chunks and VectorE chunks live in SEPARATE PSUM
    pools (psA bufs=2, psD bufs=1, psO bufs=2 -> exactly 8 banks), so the
    two engines' pipelines do not serialize through a shared slot
    rotation -- this was the v3 bottleneck (PE waited ~1us per group on
    exp with both engines only ~50% busy).
  - v-bias folds into the proj bias (out = PV/denom + bv exactly).
  - GroupNorm rstd = exp(-0.5*ln(var+eps)): only the natural_log_exp
    table set is ever loaded on ScalarE.
  - Post-division attention scaling on GpSimd; proj bias + residual in
    one scalar_tensor_tensor.
"""

import sys

import numpy as np

sys.path.insert(0, "/opt/trn_rl_repo")

B, C, HW = 4, 256, 4096
NQ = HW // 2  # queries per core
NH, HD = 4, 64
G = 32  # groups
EPS = 1e-5

# Per-(j,pair) exp split: which key chunks (0..31) VectorE handles.
# jp = 2*j + pair. Early jps keep VectorE light (it also drains the qkv
# projections); never the last chunks (they gate the psO drain).
# NOTE: pool placement (psA vs psD) is fixed by kc % 3 == 2 regardless of
# engine -- the lag-2 PE pipeline keeps 3 score tiles live, which exactly
# fits psA bufs=2 + psD bufs=1 when psD takes every third chunk.
# Engine pattern alternates so neither engine ever gets two consecutive
# chunks (a 2:1 run structure makes the faster engine the 765ns/chunk
# pacer); chunk 31 stays on ScalarE so the psO drain is not queued
# behind VectorE exps.
DVE_SCHED = {
    0: (1, 3, 7, 9, 13, 15, 19, 21, 25, 27, 29),
    1: (1, 3, 5, 7, 9, 11, 13, 15, 19, 21, 23, 25, 27, 29),
}
DVE_STEADY = (0, 2, 4, 6, 8, 10, 12, 14, 16, 18, 20, 22, 24, 26, 28, 30)

# Schraudolph constants, fp8e4m3 flavor: byte = trunc(x_raw * A8 + B8)
# approximates exp(x_raw/8 - SHIFT8) as e4m3 bits.  The softmax is
# shift-invariant, so SHIFT8 cancels exactly (the ones-column denominator
# is scaled identically); it just centers the weights in e4m3's range so
# almost nothing is subnormal or clamped.  Numerically validated: max
# softmax-output error ~5e-3 (vs out scale 0.1) with fp8 V, ~2.4x the old
# bf16 trick, still ~10x inside the rel-err gate.
LOG2E = 1.4426950408889634
SHIFT8 = 2.0
A8 = LOG2E
B8 = 56.0 - 8.0 * SHIFT8 * LOG2E

# HAM warmup dummy-matmul counts (see _build): free-running batch from
# t~6.5us, then a few per bn_stats completion to pace through the prologue.
N_WARM_FREE = 14
N_WARM_PER_STAT = 3

_CACHE = {}


def _build():
    import concourse.bass as bass
    import concourse.tile as tile
    from concourse import bacc, mybir
    from concourse.tile import add_dep_helper

    f32 = mybir.dt.float32
    bf16 = mybir.dt.bfloat16
    u8 = mybir.dt.uint8
    f8 = mybir.dt.float8e4
    AF = mybir.ActivationFunctionType
    ALU = mybir.AluOpType
    DR = mybir.MatmulPerfMode.DoubleRow

    nc = bacc.Bacc(
        "TRN2",
        target_bir_lowering=False,
        debug=False,
        enable_asserts=False,
        num_devices=8,
    )

    x_d = nc.dram_tensor("x", [C, HW], f32, kind="ExternalInput").ap()
    x_bf_d = nc.dram_tensor("x_bf", [C, HW], bf16, kind="ExternalInput").ap()
    x_f8_d = nc.dram_tensor("x_f8", [C, HW], f8, kind="ExternalInput").ap()
    qkv_wt_d = nc.dram_tensor("qkv_wt", [C, 3 * C], bf16, kind="ExternalInput").ap()
    qkv_b_d = nc.dram_tensor("qkv_b", [3 * C], f32, kind="ExternalInput").ap()
    proj_wt_d = nc.dram_tensor("proj_wt", [C, C], bf16, kind="ExternalInput").ap()
    proj_b_d = nc.dram_tensor("proj_b", [C], f32, kind="ExternalInput").ap()
    gn_w_d = nc.dram_tensor("gn_w", [C], f32, kind="ExternalInput").ap()
    gn_b_d = nc.dram_tensor("gn_b", [C], f32, kind="ExternalInput").ap()
    sel_d = nc.dram_tensor("sel", [128, 16], f32, kind="ExternalInput").ap()
    selT_d = nc.dram_tensor("selT", [16, 128], f32, kind="ExternalInput").ap()
    y_d = nc.dram_tensor("y", [C, NQ], f32, kind="ExternalOutput").ap()

    x_r = x_d.rearrange("(t p) n -> p t n", p=128)  # c = t*128 + p
    x_bf_r = x_bf_d.rearrange("(t p) n -> p t n", p=128)
    x_f8_r = x_f8_d.rearrange("(t p) n -> p t n", p=128)
    y_r = y_d.rearrange("(t p) n -> p t n", p=128)

    with tile.TileContext(nc) as tc:
        with (
            tc.tile_pool(name="const", bufs=1) as const,
            tc.tile_pool(name="big", bufs=1) as big,
            tc.tile_pool(name="work", bufs=2) as work,
            tc.tile_pool(name="pf8", bufs=6) as pf8,
            tc.tile_pool(name="psA", bufs=2, space="PSUM") as psA_pool,
            tc.tile_pool(name="psD", bufs=1, space="PSUM") as psD_pool,
            tc.tile_pool(name="psO", bufs=2, space="PSUM") as psO_pool,
            tc.tile_pool(name="dram", bufs=2, space="DRAM") as dram_pool,
        ):
            # ---- x load FIRST: everything gates on GroupNorm stats over the
            # full image, and each DMA costs ~600ns of issue time on the Sync
            # queue -- so the x transfers go ahead of all const DMAs.  Split
            # into 8 chunks so bn_stats can start on the first quarter while
            # the rest streams in.
            xb = big.tile([128, 2, HW], bf16, tag="xb")
            for t in range(2):
                for q4 in range(4):
                    nc.sync.dma_start(
                        xb[:, t, q4 * 1024 : (q4 + 1) * 1024],
                        x_bf_r[:, t, q4 * 1024 : (q4 + 1) * 1024],
                    )
            # fp8 copy of x for the qkv DoubleRow matmuls (bf16 copy above
            # still feeds the GroupNorm stats); needed from ~24us on.
            x8 = big.tile([128, 2, HW], f8, tag="x8")
            for t in range(2):
                nc.sync.dma_start(x8[:, t, :], x_f8_r[:, t, :])

            # ---- HAM warmup + activation-table preload ----
            # The PE clock gate (HAM) only un-throttles (1.2 -> 2.4 GHz)
            # after ~3.4us of sustained matmul activity, and re-throttles
            # after an idle window.  The GroupNorm-stats prologue leaves the
            # PE idle for ~15us, so the whole first attention stream used to
            # run at half clock.  Dummy matmuls (results never read) keep the
            # PE busy from t~6.5us until the real stream starts: a free-run
            # batch first, then batches paced by bn_stats completions so they
            # stretch to the end of the stats chain without delaying it.
            warm = const.tile([128, 512], bf16, tag="warm")
            nc.vector.memset(warm, 0.25)

            def dummy_mm(dep=None):
                # shares the psO tag so the pool stays at 2 PSUM banks; the
                # warmup rotation retires long before the first real psO.
                psW = psO_pool.tile([128, 512], f32, tag="psO")
                m = nc.tensor.matmul(
                    psW, lhsT=warm[:, 0:128], rhs=warm, start=True, stop=True
                )
                if dep is not None:
                    add_dep_helper(m.ins, dep.ins, sync=True, reason="warm pace")

            # The Exp activation table loads on first use (~1.3us); trigger
            # the load now while ScalarE is idle instead of on the GroupNorm
            # critical path.  Exp is the ONLY table set ever used (rstd is
            # computed on the DVE below), so there is no table switching.
            tpre = const.tile([1, 8], f32, tag="tpre")
            nc.vector.memset(tpre, 1.0)
            nc.scalar.activation(tpre, tpre, AF.Exp)
            # per-partition bias tile for the fp8 exp shift
            shift_sb = const.tile([128, 1], f32, tag="shift8")
            nc.vector.memset(shift_sb, -SHIFT8)

            for _ in range(N_WARM_FREE):
                dummy_mm()

            # ---- constants / weights ----
            wt_sb = const.tile([128, 2, 3 * C], bf16, tag="wt")
            nc.sync.dma_start(wt_sb, qkv_wt_d.rearrange("(t p) o -> p t o", p=128))
            wproj_sb = const.tile([64, NH, C], bf16, tag="wproj")
            nc.sync.dma_start(wproj_sb, proj_wt_d.rearrange("(h p) o -> p h o", p=64))
            qkvb_sb = const.tile([128, 6], f32, tag="qkvb")
            nc.sync.dma_start(qkvb_sb, qkv_b_d.rearrange("(s p) -> p s", p=128))
            vb_sb = const.tile([64, NH], f32, tag="vb")
            nc.sync.dma_start(vb_sb, qkv_b_d[2 * C :].rearrange("(h p) -> p h", p=64))
            projb_sb = const.tile([128, 2], f32, tag="projb")
            nc.sync.dma_start(projb_sb, proj_b_d.rearrange("(t p) -> p t", p=128))
            gnw_sb = const.tile([128, 2], f32, tag="gnw")
            nc.sync.dma_start(gnw_sb, gn_w_d.rearrange("(t p) -> p t", p=128))
            gnb_sb = const.tile([128, 2], f32, tag="gnb")
            nc.sync.dma_start(gnb_sb, gn_b_d.rearrange("(t p) -> p t", p=128))

            # group-selector matrices (channels<->groups), used for the tiny
            # cross-partition reductions in GroupNorm stats.
            ones1 = const.tile([1, 64], f32, tag="ones1")
            nc.vector.memset(ones1, 1.0)
            sel = const.tile([128, 16], f32, tag="sel")  # sel[p, g]=1 if p//8==g
            nc.sync.dma_start(sel, sel_d)
            selT = const.tile([16, 128], f32, tag="selT")
            nc.sync.dma_start(selT, selT_d)

            # ---- GroupNorm stats (bn_stats free dim is capped at 512) ----
            stats = const.tile([128, 2, 8, 6], f32, tag="stats")
            mv = const.tile([128, 2, 2], f32, tag="mv")
            st_handles = []
            for t in range(2):
                for jj in range(8):
                    st = nc.vector.bn_stats(
                        stats[:, t, jj, :], xb[:, t, jj * 512 : (jj + 1) * 512]
                    )
                    st_handles.append(st)
                nc.vector.bn_aggr(mv[:, t, :], stats[:, t])
            # paced warmup: a few dummy matmuls gated on each bn_stats so the
            # PE never idles past the HAM re-throttle window while the DVE
            # stats chain (the prologue critical path) runs.
            for st in st_handles:
                for _ in range(N_WARM_PER_STAT):
                    dummy_mm(dep=st)
            # me2[:, t, 0] = mean_c ; me2[:, t, 1] = E[x^2]_c = var + mean^2
            me2 = const.tile([128, 2, 2], f32, tag="me2")
            for t in range(2):
                nc.vector.tensor_copy(me2[:, t, 0:1], mv[:, t, 0:1])
                nc.vector.tensor_mul(me2[:, t, 1:2], mv[:, t, 0:1], mv[:, t, 0:1])
                nc.vector.tensor_add(me2[:, t, 1:2], me2[:, t, 1:2], mv[:, t, 1:2])
            # group sums via selector matmul (fp32 for exactness)
            psg = psA_pool.tile([16, 2, 2], f32, tag="psS")
            for t in range(2):
                nc.tensor.matmul(
                    psg[:, t, :], lhsT=sel, rhs=me2[:, t, :], start=True, stop=True
                )
            gsb = const.tile([16, 2, 2], f32, tag="gsb")
            gmr = const.tile([16, 2, 2], f32, tag="gmr")  # (mean_g, rstd_g)
            # rstd = rsqrt(var+eps) on the DVE via fast-inverse-sqrt + one
            # Newton step (rel err ~2e-3, noise vs the ~3% Schraudolph
            # ripple).  Keeping this off ScalarE means the Exp activation
            # table is the only set ever loaded -- the Ln<->Exp table
            # switches (1.3us each) this replaces sat on the critical path
            # and could thrash per-t.
            i32 = mybir.dt.int32
            magic = const.tile([16, 1], i32, tag="magic")
            nc.vector.memset(magic, 0x5F3759DF)
            rsq = const.tile([16, 2, 8], f32, tag="rsq")
            for t in range(2):
                nc.vector.tensor_scalar_mul(gsb[:, t, :], psg[:, t, :], 1.0 / 8.0)
                nc.vector.tensor_copy(gmr[:, t, 0:1], gsb[:, t, 0:1])
                # var_g = E2_g - mean_g^2
                nc.vector.tensor_mul(gmr[:, t, 1:2], gsb[:, t, 0:1], gsb[:, t, 0:1])
                nc.vector.tensor_sub(gmr[:, t, 1:2], gsb[:, t, 1:2], gmr[:, t, 1:2])
                ve = rsq[:, t, 0:1]  # var + eps
                nc.vector.tensor_scalar(
                    ve, gmr[:, t, 1:2], EPS, 0.0, ALU.add, ALU.bypass
                )
                # y0 bits = magic - (bits(ve) >> 1)
                sh = rsq[:, t, 1:2]
                nc.vector.tensor_scalar(
                    sh.bitcast(i32), ve.bitcast(i32), 1, 0,
                    ALU.logical_shift_right, ALU.bypass,
                )
                y0 = rsq[:, t, 2:3]
                nc.vector.tensor_sub(y0.bitcast(i32), magic, sh.bitcast(i32))
                # one Newton step: y1 = y0 * (1.5 - 0.5*ve*y0^2)
                aa = rsq[:, t, 3:4]
                nc.vector.tensor_mul(aa, y0, y0)
                nc.vector.tensor_mul(aa, aa, ve)
                nc.vector.tensor_scalar(aa, aa, -0.5, 1.5, ALU.mult, ALU.add)
                nc.vector.tensor_mul(gmr[:, t, 1:2], y0, aa)
            # broadcast group stats back to channels
            psb = psA_pool.tile([128, 2, 2], f32, tag="psS")
            for t in range(2):
                nc.tensor.matmul(
                    psb[:, t, :], lhsT=selT, rhs=gmr[:, t, :], start=True, stop=True
                )
            # per-channel affine: xn = x * a + bcoef  (xn in bf16 for PE)
            ab = const.tile([128, 2, 2], f32, tag="ab")
            for t in range(2):
                nc.vector.tensor_mul(ab[:, t, 0:1], psb[:, t, 1:2], gnw_sb[:, t : t + 1])
                nc.vector.tensor_mul(ab[:, t, 1:2], psb[:, t, 0:1], ab[:, t, 0:1])
                nc.vector.tensor_sub(ab[:, t, 1:2], gnb_sb[:, t : t + 1], ab[:, t, 1:2])
            # Fold the GroupNorm affine into the qkv weights instead of
            # normalizing x:  W(ax+b) + c = (Wa)x + (Wb + c).
            bvec = const.tile([128, 2, 1], bf16, tag="bvec")
            for t in range(2):
                nc.vector.tensor_copy(bvec[:, t, :], ab[:, t, 1:2])
            qkvb2 = const.tile([128, 6], f32, tag="qkvb2")
            for s in range(2):  # q 128-wide output sections (k bias cancels)
                psq = psA_pool.tile([128, 1], f32, tag="psS")
                for t in range(2):
                    nc.tensor.matmul(
                        psq,
                        lhsT=wt_sb[:, t, s * 128 : (s + 1) * 128],
                        rhs=bvec[:, t, :],
                        start=(t == 0),
                        stop=(t == 1),
                    )
                nc.vector.tensor_add(qkvb2[:, s : s + 1], qkvb_sb[:, s : s + 1], psq)
            vb2 = const.tile([64, NH], f32, tag="vb2")
            for h in range(NH):  # v bias per head (64-wide, base partition 0)
                psv = psA_pool.tile([64, 1], f32, tag="psS")
                for t in range(2):
                    nc.tensor.matmul(
                        psv,
                        lhsT=wt_sb[:, t, 2 * C + h * 64 : 2 * C + (h + 1) * 64],
                        rhs=bvec[:, t, :],
                        start=(t == 0),
                        stop=(t == 1),
                    )
                nc.vector.tensor_add(vb2[:, h : h + 1], vb_sb[:, h : h + 1], psv)
            # out = PV/denom + vb2 exactly, so the v bias folds into the proj
            # bias: projb2 = proj_b + proj_w @ vb2.
            vb2bf = const.tile([64, NH], bf16, tag="vb2bf")
            nc.vector.tensor_copy(vb2bf, vb2)
            projb2 = const.tile([128, 2], f32, tag="projb2")
            for ot in range(2):
                psB = psA_pool.tile([128, 1], f32, tag="psS")
                for h in range(NH):
                    nc.tensor.matmul(
                        psB,
                        lhsT=wproj_sb[:, h, ot * 128 : (ot + 1) * 128],
                        rhs=vb2bf[:, h : h + 1],
                        start=(h == 0),
                        stop=(h == 3),
                    )
                nc.vector.tensor_add(projb2[:, ot : ot + 1], projb_sb[:, ot : ot + 1], psB)
            # scale the weight columns in place (after the bias matmuls),
            # then cast the scaled section to fp8 for the DoubleRow qkv
            # matmuls (contraction 256 = both t-halves in one matmul at 0.5
            # cycles/column).  Only the pair-0 q/k sections are done up
            # front -- they gate the first score chunk; the rest is deferred
            # into the jp0 stream where the first exps provide cover.
            wt8 = const.tile([128, 2, 3 * C], f8, tag="wt8")

            def scale_sec(lo, hi):
                for t in range(2):
                    nc.vector.tensor_scalar_mul(
                        wt_sb[:, t, lo:hi], wt_sb[:, t, lo:hi], ab[:, t, 0:1]
                    )
                nc.vector.tensor_copy(wt8[:, :, lo:hi], wt_sb[:, :, lo:hi])

            scale_sec(0, 128)
            scale_sec(C, C + 128)

            def scale_rest():
                scale_sec(128, 256)
                scale_sec(C + 128, C + 256)
                scale_sec(2 * C, 3 * C)

            # ---- QKV projections ----
            # k2/q2: [128, pair, n] with head (2*pair + p//64) at partition
            # (p%64); produced directly by 128-wide output matmuls.
            k2 = big.tile([128, 2, HW], bf16, tag="k2")
            q2 = big.tile([128, 2, NQ], bf16, tag="q2")
            # vT8: [key_chunk_part, kc_pair, parity, h, 72] fp8e4m3; col 64 =
            # ones (denominator).  The (parity, h, 0:65) slice is the
            # DoubleRow lhsT [Ki=128, Ko=2, M=65]; dim padded 65->72 so the
            # Ko step (NH*72 = 288 B) is 16B-aligned as DoubleRow requires.
            vT8 = big.tile([128, 16, 2, NH, 72], f8, tag="vT8")
            onesc = const.tile([128, 1], f32, tag="onesc")
            nc.vector.memset(onesc, 1.0)
            nc.vector.tensor_copy(
                vT8[:, :, :, :, 64:65], onesc.to_broadcast((128, 16, 2, NH, 1))
            )

            def emit_q(pair, j4s):
                for j4 in j4s:
                    ps = psA_pool.tile([128, 512], f32, tag="psS")
                    nc.tensor.matmul(
                        ps,
                        lhsT=wt8[:, :, pair * 128 : (pair + 1) * 128],
                        rhs=x8[:, :, j4 * 512 : (j4 + 1) * 512],
                        start=True,
                        stop=True,
                        perf_mode=DR,
                    )
                    nc.vector.tensor_scalar_add(
                        q2[:, pair, j4 * 512 : (j4 + 1) * 512], ps,
                        qkvb2[:, pair : pair + 1],
                    )

            def emit_k(pair, n8s):
                # NOTE: the k bias is dropped entirely -- q.(Wk x + bk) =
                # q.Wk x + q.bk, and q.bk is constant across keys, so it
                # cancels in the softmax.  The PSUM->SBUF drain is a pure
                # copy and runs on ScalarE (Copy is in every activation
                # table set), rebalancing jp0's drain load off the DVE.
                for n8 in n8s:
                    ps = psA_pool.tile([128, 512], f32, tag="psS")
                    nc.tensor.matmul(
                        ps,
                        lhsT=wt8[:, :, C + pair * 128 : C + (pair + 1) * 128],
                        rhs=x8[:, :, n8 * 512 : (n8 + 1) * 512],
                        start=True,
                        stop=True,
                        perf_mode=DR,
                    )
                    nc.scalar.activation(
                        k2[:, pair, n8 * 512 : (n8 + 1) * 512], ps, AF.Copy
                    )

            def emit_vT(kcps):
                # one [128, 512] PSUM tile + ONE strided drain per chunk
                # PAIR (vs per chunk): halves the DVE drain instruction count
                for kcp in kcps:
                    ps = psA_pool.tile([128, 2, 256], f32, tag="psS")
                    for par in range(2):
                        nc.tensor.matmul(
                            ps[:, par, :],
                            lhsT=x8[:, :, (2 * kcp + par) * 128
                                    : (2 * kcp + par + 1) * 128],
                            rhs=wt8[:, :, 2 * C : 3 * C],
                            start=True,
                            stop=True,
                            perf_mode=DR,
                        )
                    nc.vector.tensor_copy(
                        vT8[:, kcp, :, :, 0:64],
                        ps.rearrange("p x (h d) -> p x h d", h=NH),
                    )

            # minimal prefix before attention starts: one k tile + one q tile
            # (chunks 0-3 only need k-tile 0).  Everything else -- remaining
            # weight-section folds, k tiles, vT chunks, pair-1 qkv -- streams
            # through jp0's deferred slots, paced just ahead of consumption,
            # so the first exp starts ~15us earlier.
            emit_k(0, [0, 1])
            emit_q(0, [0])
            # deferred emission schedule: {jp: {cyc: thunk}}; k tile n8 is
            # consumed from chunk 4*n8, emitted >=5 cycles earlier so its
            # drain never gates a score.
            deferred = {
                0: {
                    0: lambda: (scale_rest(), emit_vT(range(0, 2))),
                    1: lambda: (emit_k(0, [2]), emit_vT(range(2, 4))),
                    4: lambda: (emit_k(0, [3]), emit_vT(range(4, 6))),
                    7: lambda: (emit_k(0, [4]), emit_vT(range(6, 8))),
                    10: lambda: (emit_k(0, [5]), emit_vT(range(8, 10))),
                    13: lambda: (emit_k(0, [6]), emit_vT(range(10, 12))),
                    16: lambda: (emit_k(0, [7]), emit_vT(range(12, 14))),
                    19: lambda: emit_vT(range(14, 16)),
                    22: lambda: emit_k(1, range(0, 4)),
                    25: lambda: emit_k(1, range(4, 8)),
                    28: lambda: emit_q(1, [0]),
                },
                1: {3: lambda: emit_q(0, [1]), 13: lambda: emit_q(1, [1])},
                2: {3: lambda: emit_q(0, [2]), 13: lambda: emit_q(1, [2])},
                3: {3: lambda: emit_q(0, [3]), 13: lambda: emit_q(1, [3])},
            }

            def emit_xres(j):
                # residual fetch for query tile j, issued well before the
                # proj needs it so the DMA latency hides
                xres = work.tile([128, 2, 512], f32, tag="xres")
                nc.sync.dma_start(xres, x_r[:, :, j * 512 : (j + 1) * 512])
                return xres

            def emit_proj(j, att_j, xres):
                # proj + bias + residual for query tile j (emitted lazily so
                # the in-order PE stream never stalls on the division tail)
                y_sb = work.tile([128, 2, 512], f32, tag="y")
                for ot in range(2):
                    psY = psO_pool.tile([128, 512], f32, tag="psO")
                    for h in range(4):
                        nc.tensor.matmul(
                            psY,
                            lhsT=wproj_sb[:, h, ot * 128 : (ot + 1) * 128],
                            rhs=att_j[:, h, :],
                            start=(h == 0),
                            stop=(h == 3),
                        )
                    # y = (psY + projb2) + xres in one DVE op
                    nc.vector.scalar_tensor_tensor(
                        y_sb[:, ot, :], psY, projb2[:, ot : ot + 1],
                        xres[:, ot, :], ALU.add, ALU.add,
                    )
                nc.sync.dma_start(y_r[:, :, j * 512 : (j + 1) * 512], y_sb)

            pending = None
            for j in range(4):
                att_j = work.tile([64, NH, 512], bf16, tag="att")
                for pair in range(2):
                    jp = 2 * j + pair
                    dve_kcs = DVE_SCHED.get(jp, DVE_STEADY)
                    defer_jp = deferred.get(jp, {})
                    psO0 = psO_pool.tile([65, 512], f32, tag="psO")
                    psO1 = psO_pool.tile([65, 512], f32, tag="psO")
                    prev_st = None
                    live_P = {}
                    # lag-4 software pipeline on the in-order PE stream:
                    # cycle c emits scores(c) then PV(c-4).  The score tiles
                    # are freed by their exp (not the PV), so the PV lag is
                    # free to be deep -- by lag 4 the exp is always done and
                    # the in-order PE queue never stalls on a PV semaphore.
                    for cyc in range(36):
                        if cyc < 32:
                            kc = cyc
                            # scores for both heads of the pair in one tile:
                            # S[:, 0, :] head 2p (PE rows 0-63), S[:, 1, :]
                            # head 2p+1 (rows 64-127); the two matmuls are
                            # pc-adjacent so they overlap in the PE array.
                            S = (psD_pool if kc % 3 == 2 else psA_pool).tile(
                                [128, 2, 512], f32,
                                tag="psD" if kc % 3 == 2 else "psS",
                            )
                            ma = nc.tensor.matmul(
                                S[:, 0, :],
                                lhsT=k2[0:64, pair, kc * 128 : (kc + 1) * 128],
                                rhs=q2[0:64, pair, j * 512 : (j + 1) * 512],
                                start=True,
                                stop=True,
                            )
                            mb = nc.tensor.matmul(
                                S[:, 1, :],
                                lhsT=k2[64:128, pair, kc * 128 : (kc + 1) * 128],
                                rhs=q2[64:128, pair, j * 512 : (j + 1) * 512],
                                start=True,
                                stop=True,
                            )
                            if prev_st is not None:
                                add_dep_helper(
                                    ma.ins, prev_st, sync=False,
                                    reason="st-pair order",
                                )
                            add_dep_helper(
                                mb.ins, ma.ins, sync=False, reason="st-pair order"
                            )
                            prev_st = mb.ins
                            # exp writes e4m3 into the chunk-pair P tile:
                            # parity 0 (even kc, ScalarE exact exp w/ RNE
                            # cast) or parity 1 (odd kc, VectorE Schraudolph
                            # bits as uint8) -- the engines write disjoint
                            # slices concurrently.
                            if kc % 2 == 0:
                                Pp = pf8.tile([128, 2, 2, 512], f8, tag="Pp")
                                live_P[kc // 2] = Pp
                            else:
                                Pp = live_P[kc // 2]
                            par = kc % 2
                            if kc in dve_kcs:
                                nc.vector.tensor_scalar(
                                    Pp[:, par, :, :].bitcast(u8), S,
                                    A8, B8, ALU.mult, ALU.add,
                                )
                            else:
                                nc.scalar.activation(
                                    Pp[:, par, :, :], S, AF.Exp,
                                    bias=shift_sb, scale=float(HD) ** -0.5,
                                )
                        if cyc >= 5 and (cyc - 5) % 2 == 0:
                            # fp8 DoubleRow PV: one matmul per head covers a
                            # chunk PAIR (contraction 256 = 2 fp8 weights per
                            # PE cell, 0.5 cycles/column) -- halves the PE
                            # time of the PV side.
                            m = (cyc - 5) // 2
                            Pp = live_P.pop(m)
                            for hp, psO in ((0, psO0), (1, psO1)):
                                nc.tensor.matmul(
                                    psO,
                                    lhsT=vT8[:, m, :, 2 * pair + hp, 0:65],
                                    rhs=Pp[:, :, hp, :],
                                    start=(m == 0),
                                    stop=(m == 15),
                                    perf_mode=DR,
                                )
                        if cyc in defer_jp:
                            defer_jp[cyc]()
                        # emit_proj is deferred to cyc 14 of the next pair:
                        # the division chain for att_j (drain -> DMA spread /
                        # recip / broadcast -> gpsimd mul) takes ~10us after
                        # the last PV, and the proj matmuls sit in the
                        # in-order PE stream -- emitted too early they dam up
                        # everything behind them (scores AND the PV-DRs whose
                        # retirement the exp-engine P-tile rotation waits on,
                        # stalling the whole pair ~6.5us).  The xres fetch
                        # has no att dependency and issues at cyc 0.
                        if pending is not None and pair == 0:
                            if cyc == 0:
                                pending = pending + (emit_xres(pending[0]),)
                            elif cyc == 14:
                                emit_proj(*pending)
                                pending = None
                    # copy out of PSUM right away so the psO slots free for
                    # the next head pair; the division chain below works off
                    # the SBUF copy, off the critical path.  Both heads drain
                    # into one [65, 1024] SBUF tile so the reciprocal /
                    # broadcast chain runs once per pair instead of per head.
                    oc = work.tile([65, 1024], f32, tag="oc")
                    nc.scalar.activation(oc[:, 0:512], psO0, AF.Copy)
                    nc.scalar.activation(oc[:, 512:1024], psO1, AF.Copy)
                    # spread the denominator row over 64 partitions so the
                    # iterative DVE reciprocal (8 cyc/elem) is not
                    # single-lane-bound; the broadcast bounces through DRAM
                    # (SBUF DMA sources cannot have a zero partition step,
                    # and GpSimd-side alternatives measured slower).
                    r8 = work.tile([64, 16], f32, tag="r8")
                    nc.sync.dma_start(
                        r8, oc[64:65, :].rearrange("o (a b) -> o a b", b=16)
                    )
                    nc.vector.reciprocal(r8, r8)
                    rec_d = dram_pool.tile([1, 1024], f32, tag="recd")
                    nc.sync.dma_start(
                        rec_d.rearrange("o (a b) -> o a b", b=16), r8
                    )
                    rec_b = work.tile([64, 1024], f32, tag="recb")
                    nc.sync.dma_start(
                        rec_b, rec_d[0:1, :].to_broadcast((64, 1024))
                    )
                    for hp in (0, 1):
                        h = 2 * pair + hp
                        # attention scaling on GpSimd (otherwise idle)
                        nc.gpsimd.tensor_mul(
                            att_j[:, h, :],
                            oc[0:64, hp * 512 : (hp + 1) * 512],
                            rec_b[:, hp * 512 : (hp + 1) * 512],
                        )
                pending = (j, att_j)
            emit_proj(*pending, emit_xres(pending[0]))

    nc.compile()
    return nc


def _get_program():
    if "nc" not in _CACHE:
        _CACHE["nc"] = _build()
    return _CACHE["nc"]


def kernel(x, gn_w, gn_b, qkv_w, qkv_b, proj_w, proj_b):
    import ml_dtypes

    from concourse.bass_utils import run_bass_kernel_spmd

    x = np.asarray(x, np.float32)
    gn_w = np.asarray(gn_w, np.float32)
    gn_b = np.asarray(gn_b, np.float32)
    qkv_w = np.asarray(qkv_w, np.float32)
    qkv_b = np.asarray(qkv_b, np.float32)
    proj_w = np.asarray(proj_w, np.float32)
    proj_b = np.asarray(proj_b, np.float32)

    nc = _get_program()
    qkv_wt = np.ascontiguousarray(qkv_w.T).astype(ml_dtypes.bfloat16)
    proj_wt = np.ascontiguousarray(proj_w.T).astype(ml_dtypes.bfloat16)
    sel = np.zeros((128, 16), np.float32)
    sel[np.arange(128), np.arange(128) // 8] = 1.0
    selT = np.ascontiguousarray(sel.T)

    in_maps = []
    for core in range(8):
        b, half = core // 2, core % 2
        xb = x[b].reshape(C, HW)
        if half == 1:
            xb = np.concatenate([xb[:, NQ:], xb[:, :NQ]], axis=1)
        in_maps.append(
            {
                "x": np.ascontiguousarray(xb),
                "x_bf": np.ascontiguousarray(xb).astype(ml_dtypes.bfloat16),
                "x_f8": np.ascontiguousarray(xb).astype(ml_dtypes.float8_e4m3fn),
                "qkv_wt": qkv_wt,
                "qkv_b": qkv_b,
                "proj_wt": proj_wt,
                "proj_b": proj_b,
                "gn_w": gn_w,
                "gn_b": gn_b,
                "sel": sel,
                "selT": selT,
            }
        )

    res = run_bass_kernel_spmd(nc, in_maps, core_ids=list(range(8)))
    out = np.empty((B, C, HW), np.float32)
    for core in range(8):
        b, half = core // 2, core % 2
        out[b][:, half * NQ : (half + 1) * NQ] = res.results[core]["y"]
    return out.reshape(B, C, 64, 64)



# revision 63
# speedup vs baseline: 1.0199x; 1.0097x over previous
"""AttentionBlock kernel for 8 Trainium2 NeuronCores (v10, ~255us HW).

Problem (hardcoded shapes): x [4, 256, 64, 64] f32.
  GroupNorm(32 groups) -> qkv 1x1 conv (768x256) -> 4-head attention over
  n=4096 tokens (hd=64) -> proj 1x1 conv -> residual add.

Sharding: 8 cores = (batch b in 0..3) x (query-half in 0..1).  Each core
computes GroupNorm + K/V for its whole batch image (duplicated across the
2 cores of a batch, cheap) and attention + proj + residual for its half of
the query positions (2048 of 4096).  Key order is permuted so the core's
query slice comes first; softmax is permutation-invariant over keys.

v10 structure (vs the 327us v4 baseline; all deltas HW-measured):
  - fp8e4m3 DoubleRow PV matmuls: each PV covers a chunk PAIR (contraction
    256 = 2 fp8 weights/PE cell, 0.5 cyc/col) -- halves the PV side of the
    PE stream, which paces the steady state.  P comes from ScalarE exact
    exp (even chunks, fp8 out) / VectorE uint8 Schraudolph (odd chunks);
    exp args are shifted by SHIFT8 (cancels in softmax) to center weights
    in e4m3 range.  V is cast to fp8 at drain time.
  - fp8 DoubleRow qkv projections too (x8/wt8 operands; GroupNorm stats
    still use the bf16 copy; GroupNorm affine folded into wt8 on-chip).
  - k-bias dropped entirely: q.(Wk x + bk) adds a per-query constant to
    scores, which softmax cancels.  k2 drains are pure ScalarE copies,
    balancing jp0's drain load across both act engines.
  - HAM management: the PE clock gate starts throttled (1.2 vs 2.4 GHz)
    and re-throttles after idle windows.  Dummy matmuls (free-running +
    bn_stats-paced) keep the PE busy through the GroupNorm prologue; the
    steady stream then holds one continuous ~200us warm window.
  - GroupNorm rstd via fast-inverse-sqrt + 1 Newton step on the DVE: the
    Exp activation table is the only set ScalarE ever loads (a Ln<->Exp
    switch costs 1.3us and thrashed per-t when scheduled naively).
  - emit_proj deferred to cyc 14 of the NEXT pair: the att division chain
    (drain -> DMA spread -> recip -> DRAM-bounce broadcast -> GpSimd mul)
    is ~10us of DMA latency; proj matmuls emitted earlier dam up the
    in-order PE stream (and the P-tile pool rotation the exps wait on),
    which cost ~6.5us + a HAM re-throttle per j boundary.
  - Softmax exp split ~16/16 between ScalarE (exact) and VectorE
    (Schraudolph); both land ~88% busy, just under the PE pace.
"""

import sys

import numpy as np

sys.path.insert(0, "/opt/trn_rl_repo")

B, C, HW = 4, 256, 4096
NQ = HW // 2  # queries per core
NH, HD = 4, 64
G = 32  # groups
EPS = 1e-5

# Per-(j,pair) exp split: which key chunks (0..31) VectorE handles.
# jp = 2*j + pair. Early jps keep VectorE light (it also drains the qkv
# projections); never the last chunks (they gate the psO drain).
# NOTE: pool placement (psA vs psD) is fixed by kc % 3 == 2 regardless of
# engine -- the lag-2 PE pipeline keeps 3 score tiles live, which exactly
# fits psA bufs=2 + psD bufs=1 when psD takes every third chunk.
# Engine pattern alternates so neither engine ever gets two consecutive
# chunks (a 2:1 run structure makes the faster engine the 765ns/chunk
# pacer); chunk 31 stays on ScalarE so the psO drain is not queued
# behind VectorE exps.
DVE_SCHED = {
    0: (1, 3, 7, 9, 13, 15, 19, 21, 25, 27, 29),
    1: (1, 3, 5, 7, 9, 11, 13, 15, 17, 19, 21, 23, 25, 27, 29),
}
DVE_STEADY = (0, 2, 4, 6, 8, 10, 12, 14, 16, 18, 20, 22, 24, 26, 28, 30)

# Schraudolph constants, fp8e4m3 flavor: byte = trunc(x_raw * A8 + B8)
# approximates exp(x_raw/8 - SHIFT8) as e4m3 bits.  The softmax is
# shift-invariant, so SHIFT8 cancels exactly (the ones-column denominator
# is scaled identically); it just centers the weights in e4m3's range so
# almost nothing is subnormal or clamped.  Numerically validated: max
# softmax-output error ~5e-3 (vs out scale 0.1) with fp8 V, ~2.4x the old
# bf16 trick, still ~10x inside the rel-err gate.
LOG2E = 1.4426950408889634
SHIFT8 = 2.0
A8 = LOG2E
B8 = 56.0 - 8.0 * SHIFT8 * LOG2E

# HAM warmup dummy-matmul counts (see _build): free-running batch from
# t~6.5us, then a few per bn_stats completion to pace through the prologue.
N_WARM_FREE = 14
N_WARM_PER_STAT = 3

_CACHE = {}


def _build():
    import concourse.bass as bass
    import concourse.tile as tile
    from concourse import bacc, mybir
    from concourse.tile import add_dep_helper

    f32 = mybir.dt.float32
    bf16 = mybir.dt.bfloat16
    u8 = mybir.dt.uint8
    f8 = mybir.dt.float8e4
    AF = mybir.ActivationFunctionType
    ALU = mybir.AluOpType
    DR = mybir.MatmulPerfMode.DoubleRow

    nc = bacc.Bacc(
        "TRN2",
        target_bir_lowering=False,
        debug=False,
        enable_asserts=False,
        num_devices=8,
    )

    x_d = nc.dram_tensor("x", [C, HW], f32, kind="ExternalInput").ap()
    x_bf_d = nc.dram_tensor("x_bf", [C, HW], bf16, kind="ExternalInput").ap()
    x_f8_d = nc.dram_tensor("x_f8", [C, HW], f8, kind="ExternalInput").ap()
    qkv_wt_d = nc.dram_tensor("qkv_wt", [C, 3 * C], bf16, kind="ExternalInput").ap()
    qkv_b_d = nc.dram_tensor("qkv_b", [3 * C], f32, kind="ExternalInput").ap()
    proj_wt_d = nc.dram_tensor("proj_wt", [C, C], bf16, kind="ExternalInput").ap()
    proj_b_d = nc.dram_tensor("proj_b", [C], f32, kind="ExternalInput").ap()
    gn_w_d = nc.dram_tensor("gn_w", [C], f32, kind="ExternalInput").ap()
    gn_b_d = nc.dram_tensor("gn_b", [C], f32, kind="ExternalInput").ap()
    sel_d = nc.dram_tensor("sel", [128, 16], f32, kind="ExternalInput").ap()
    selT_d = nc.dram_tensor("selT", [16, 128], f32, kind="ExternalInput").ap()
    y_d = nc.dram_tensor("y", [C, NQ], f32, kind="ExternalOutput").ap()

    x_r = x_d.rearrange("(t p) n -> p t n", p=128)  # c = t*128 + p
    x_bf_r = x_bf_d.rearrange("(t p) n -> p t n", p=128)
    x_f8_r = x_f8_d.rearrange("(t p) n -> p t n", p=128)
    y_r = y_d.rearrange("(t p) n -> p t n", p=128)

    with tile.TileContext(nc) as tc:
        with (
            tc.tile_pool(name="const", bufs=1) as const,
            tc.tile_pool(name="big", bufs=1) as big,
            tc.tile_pool(name="work", bufs=2) as work,
            tc.tile_pool(name="pf8", bufs=6) as pf8,
            tc.tile_pool(name="psA", bufs=2, space="PSUM") as psA_pool,
            tc.tile_pool(name="psD", bufs=1, space="PSUM") as psD_pool,
            tc.tile_pool(name="psO", bufs=2, space="PSUM") as psO_pool,
            tc.tile_pool(name="dram", bufs=2, space="DRAM") as dram_pool,
        ):
            # ---- x load FIRST: everything gates on GroupNorm stats over the
            # full image, and each DMA costs ~600ns of issue time on the Sync
            # queue -- so the x transfers go ahead of all const DMAs.  Split
            # into 8 chunks so bn_stats can start on the first quarter while
            # the rest streams in.
            xb = big.tile([128, 2, HW], bf16, tag="xb")
            for t in range(2):
                for q4 in range(4):
                    nc.sync.dma_start(
                        xb[:, t, q4 * 1024 : (q4 + 1) * 1024],
                        x_bf_r[:, t, q4 * 1024 : (q4 + 1) * 1024],
                    )
            # fp8 copy of x for the qkv DoubleRow matmuls (bf16 copy above
            # still feeds the GroupNorm stats); needed from ~24us on.
            x8 = big.tile([128, 2, HW], f8, tag="x8")
            for t in range(2):
                nc.sync.dma_start(x8[:, t, :], x_f8_r[:, t, :])

            # ---- HAM warmup + activation-table preload ----
            # The PE clock gate (HAM) only un-throttles (1.2 -> 2.4 GHz)
            # after ~3.4us of sustained matmul activity, and re-throttles
            # after an idle window.  The GroupNorm-stats prologue leaves the
            # PE idle for ~15us, so the whole first attention stream used to
            # run at half clock.  Dummy matmuls (results never read) keep the
            # PE busy from t~6.5us until the real stream starts: a free-run
            # batch first, then batches paced by bn_stats completions so they
            # stretch to the end of the stats chain without delaying it.
            warm = const.tile([128, 512], bf16, tag="warm")
            nc.vector.memset(warm, 0.25)

            def dummy_mm(dep=None):
                # shares the psO tag so the pool stays at 2 PSUM banks; the
                # warmup rotation retires long before the first real psO.
                psW = psO_pool.tile([128, 512], f32, tag="psO")
                m = nc.tensor.matmul(
                    psW, lhsT=warm[:, 0:128], rhs=warm, start=True, stop=True
                )
                if dep is not None:
                    add_dep_helper(m.ins, dep.ins, sync=True, reason="warm pace")

            # The Exp activation table loads on first use (~1.3us); trigger
            # the load now while ScalarE is idle instead of on the GroupNorm
            # critical path.  Exp is the ONLY table set ever used (rstd is
            # computed on the DVE below), so there is no table switching.
            tpre = const.tile([1, 8], f32, tag="tpre")
            nc.vector.memset(tpre, 1.0)
            nc.scalar.activation(tpre, tpre, AF.Exp)
            # per-partition bias tile for the fp8 exp shift
            shift_sb = const.tile([128, 1], f32, tag="shift8")
            nc.vector.memset(shift_sb, -SHIFT8)

            for _ in range(N_WARM_FREE):
                dummy_mm()

            # ---- constants / weights ----
            wt_sb = const.tile([128, 2, 3 * C], bf16, tag="wt")
            nc.sync.dma_start(wt_sb, qkv_wt_d.rearrange("(t p) o -> p t o", p=128))
            wproj_sb = const.tile([64, NH, C], bf16, tag="wproj")
            nc.sync.dma_start(wproj_sb, proj_wt_d.rearrange("(h p) o -> p h o", p=64))
            qkvb_sb = const.tile([128, 6], f32, tag="qkvb")
            nc.sync.dma_start(qkvb_sb, qkv_b_d.rearrange("(s p) -> p s", p=128))
            vb_sb = const.tile([64, NH], f32, tag="vb")
            nc.sync.dma_start(vb_sb, qkv_b_d[2 * C :].rearrange("(h p) -> p h", p=64))
            projb_sb = const.tile([128, 2], f32, tag="projb")
            nc.sync.dma_start(projb_sb, proj_b_d.rearrange("(t p) -> p t", p=128))
            gnw_sb = const.tile([128, 2], f32, tag="gnw")
            nc.sync.dma_start(gnw_sb, gn_w_d.rearrange("(t p) -> p t", p=128))
            gnb_sb = const.tile([128, 2], f32, tag="gnb")
            nc.sync.dma_start(gnb_sb, gn_b_d.rearrange("(t p) -> p t", p=128))

            # group-selector matrices (channels<->groups), used for the tiny
            # cross-partition reductions in GroupNorm stats.
            ones1 = const.tile([1, 64], f32, tag="ones1")
            nc.vector.memset(ones1, 1.0)
            sel = const.tile([128, 16], f32, tag="sel")  # sel[p, g]=1 if p//8==g
            nc.sync.dma_start(sel, sel_d)
            selT = const.tile([16, 128], f32, tag="selT")
            nc.sync.dma_start(selT, selT_d)

            # ---- GroupNorm stats (bn_stats free dim is capped at 512) ----
            stats = const.tile([128, 2, 8, 6], f32, tag="stats")
            mv = const.tile([128, 2, 2], f32, tag="mv")
            st_handles = []
            for t in range(2):
                for jj in range(8):
                    st = nc.vector.bn_stats(
                        stats[:, t, jj, :], xb[:, t, jj * 512 : (jj + 1) * 512]
                    )
                    st_handles.append(st)
                nc.vector.bn_aggr(mv[:, t, :], stats[:, t])
            # paced warmup: a few dummy matmuls gated on each bn_stats so the
            # PE never idles past the HAM re-throttle window while the DVE
            # stats chain (the prologue critical path) runs.
            for st in st_handles:
                for _ in range(N_WARM_PER_STAT):
                    dummy_mm(dep=st)
            # me2[:, t, 0] = mean_c ; me2[:, t, 1] = E[x^2]_c = var + mean^2
            me2 = const.tile([128, 2, 2], f32, tag="me2")
            for t in range(2):
                nc.vector.tensor_copy(me2[:, t, 0:1], mv[:, t, 0:1])
                nc.vector.tensor_mul(me2[:, t, 1:2], mv[:, t, 0:1], mv[:, t, 0:1])
                nc.vector.tensor_add(me2[:, t, 1:2], me2[:, t, 1:2], mv[:, t, 1:2])
            # group sums via selector matmul (fp32 for exactness)
            psg = psA_pool.tile([16, 2, 2], f32, tag="psS")
            for t in range(2):
                nc.tensor.matmul(
                    psg[:, t, :], lhsT=sel, rhs=me2[:, t, :], start=True, stop=True
                )
            gsb = const.tile([16, 2, 2], f32, tag="gsb")
            gmr = const.tile([16, 2, 2], f32, tag="gmr")  # (mean_g, rstd_g)
            # rstd = rsqrt(var+eps) on the DVE via fast-inverse-sqrt + one
            # Newton step (rel err ~2e-3, noise vs the ~3% Schraudolph
            # ripple).  Keeping this off ScalarE means the Exp activation
            # table is the only set ever loaded -- the Ln<->Exp table
            # switches (1.3us each) this replaces sat on the critical path
            # and could thrash per-t.
            i32 = mybir.dt.int32
            magic = const.tile([16, 1], i32, tag="magic")
            nc.vector.memset(magic, 0x5F3759DF)
            rsq = const.tile([16, 2, 8], f32, tag="rsq")
            for t in range(2):
                nc.vector.tensor_scalar_mul(gsb[:, t, :], psg[:, t, :], 1.0 / 8.0)
                nc.vector.tensor_copy(gmr[:, t, 0:1], gsb[:, t, 0:1])
                # var_g = E2_g - mean_g^2
                nc.vector.tensor_mul(gmr[:, t, 1:2], gsb[:, t, 0:1], gsb[:, t, 0:1])
                nc.vector.tensor_sub(gmr[:, t, 1:2], gsb[:, t, 1:2], gmr[:, t, 1:2])
                ve = rsq[:, t, 0:1]  # var + eps
                nc.vector.tensor_scalar(
                    ve, gmr[:, t, 1:2], EPS, 0.0, ALU.add, ALU.bypass
                )
                # y0 bits = magic - (bits(ve) >> 1)
                sh = rsq[:, t, 1:2]
                nc.vector.tensor_scalar(
                    sh.bitcast(i32), ve.bitcast(i32), 1, 0,
                    ALU.logical_shift_right, ALU.bypass,
                )
                y0 = rsq[:, t, 2:3]
                nc.vector.tensor_sub(y0.bitcast(i32), magic, sh.bitcast(i32))
                # one Newton step: y1 = y0 * (1.5 - 0.5*ve*y0^2)
                aa = rsq[:, t, 3:4]
                nc.vector.tensor_mul(aa, y0, y0)
                nc.vector.tensor_mul(aa, aa, ve)
                nc.vector.tensor_scalar(aa, aa, -0.5, 1.5, ALU.mult, ALU.add)
                nc.vector.tensor_mul(gmr[:, t, 1:2], y0, aa)
            # broadcast group stats back to channels
            psb = psA_pool.tile([128, 2, 2], f32, tag="psS")
            for t in range(2):
                nc.tensor.matmul(
                    psb[:, t, :], lhsT=selT, rhs=gmr[:, t, :], start=True, stop=True
                )
            # per-channel affine: xn = x * a + bcoef  (xn in bf16 for PE)
            ab = const.tile([128, 2, 2], f32, tag="ab")
            for t in range(2):
                nc.vector.tensor_mul(ab[:, t, 0:1], psb[:, t, 1:2], gnw_sb[:, t : t + 1])
                nc.vector.tensor_mul(ab[:, t, 1:2], psb[:, t, 0:1], ab[:, t, 0:1])
                nc.vector.tensor_sub(ab[:, t, 1:2], gnb_sb[:, t : t + 1], ab[:, t, 1:2])
            # Fold the GroupNorm affine into the qkv weights instead of
            # normalizing x:  W(ax+b) + c = (Wa)x + (Wb + c).
            bvec = const.tile([128, 2, 1], bf16, tag="bvec")
            for t in range(2):
                nc.vector.tensor_copy(bvec[:, t, :], ab[:, t, 1:2])
            qkvb2 = const.tile([128, 6], f32, tag="qkvb2")
            for s in range(2):  # q 128-wide output sections (k bias cancels)
                psq = psA_pool.tile([128, 1], f32, tag="psS")
                for t in range(2):
                    nc.tensor.matmul(
                        psq,
                        lhsT=wt_sb[:, t, s * 128 : (s + 1) * 128],
                        rhs=bvec[:, t, :],
                        start=(t == 0),
                        stop=(t == 1),
                    )
                nc.vector.tensor_add(qkvb2[:, s : s + 1], qkvb_sb[:, s : s + 1], psq)
            vb2 = const.tile([64, NH], f32, tag="vb2")
            for h in range(NH):  # v bias per head (64-wide, base partition 0)
                psv = psA_pool.tile([64, 1], f32, tag="psS")
                for t in range(2):
                    nc.tensor.matmul(
                        psv,
                        lhsT=wt_sb[:, t, 2 * C + h * 64 : 2 * C + (h + 1) * 64],
                        rhs=bvec[:, t, :],
                        start=(t == 0),
                        stop=(t == 1),
                    )
                nc.vector.tensor_add(vb2[:, h : h + 1], vb_sb[:, h : h + 1], psv)
            # out = PV/denom + vb2 exactly, so the v bias folds into the proj
            # bias: projb2 = proj_b + proj_w @ vb2.
            vb2bf = const.tile([64, NH], bf16, tag="vb2bf")
            nc.vector.tensor_copy(vb2bf, vb2)
            projb2 = const.tile([128, 2], f32, tag="projb2")
            for ot in range(2):
                psB = psA_pool.tile([128, 1], f32, tag="psS")
                for h in range(NH):
                    nc.tensor.matmul(
                        psB,
                        lhsT=wproj_sb[:, h, ot * 128 : (ot + 1) * 128],
                        rhs=vb2bf[:, h : h + 1],
                        start=(h == 0),
                        stop=(h == 3),
                    )
                nc.vector.tensor_add(projb2[:, ot : ot + 1], projb_sb[:, ot : ot + 1], psB)
            # scale the weight columns in place (after the bias matmuls),
            # then cast the scaled section to fp8 for the DoubleRow qkv
            # matmuls (contraction 256 = both t-halves in one matmul at 0.5
            # cycles/column).  Only the pair-0 q/k sections are done up
            # front -- they gate the first score chunk; the rest is deferred
            # into the jp0 stream where the first exps provide cover.
            wt8 = const.tile([128, 2, 3 * C], f8, tag="wt8")

            def scale_sec(lo, hi):
                for t in range(2):
                    nc.vector.tensor_scalar_mul(
                        wt_sb[:, t, lo:hi], wt_sb[:, t, lo:hi], ab[:, t, 0:1]
                    )
                nc.vector.tensor_copy(wt8[:, :, lo:hi], wt_sb[:, :, lo:hi])

            scale_sec(0, 128)
            scale_sec(C, C + 128)

            def scale_rest():
                scale_sec(128, 256)
                scale_sec(C + 128, C + 256)
                scale_sec(2 * C, 3 * C)

            # ---- QKV projections ----
            # k2/q2: [128, pair, n] with head (2*pair + p//64) at partition
            # (p%64); produced directly by 128-wide output matmuls.
            k2 = big.tile([128, 2, HW], bf16, tag="k2")
            q2 = big.tile([128, 2, NQ], bf16, tag="q2")
            # vT8: [key_chunk_part, kc_pair, parity, h, 72] fp8e4m3; col 64 =
            # ones (denominator).  The (parity, h, 0:65) slice is the
            # DoubleRow lhsT [Ki=128, Ko=2, M=65]; dim padded 65->72 so the
            # Ko step (NH*72 = 288 B) is 16B-aligned as DoubleRow requires.
            vT8 = big.tile([128, 16, 2, NH, 72], f8, tag="vT8")
            onesc = const.tile([128, 1], f32, tag="onesc")
            nc.vector.memset(onesc, 1.0)
            nc.vector.tensor_copy(
                vT8[:, :, :, :, 64:65], onesc.to_broadcast((128, 16, 2, NH, 1))
            )

            def emit_q(pair, j4s):
                for j4 in j4s:
                    ps = psA_pool.tile([128, 512], f32, tag="psS")
                    nc.tensor.matmul(
                        ps,
                        lhsT=wt8[:, :, pair * 128 : (pair + 1) * 128],
                        rhs=x8[:, :, j4 * 512 : (j4 + 1) * 512],
                        start=True,
                        stop=True,
                        perf_mode=DR,
                    )
                    nc.vector.tensor_scalar_add(
                        q2[:, pair, j4 * 512 : (j4 + 1) * 512], ps,
                        qkvb2[:, pair : pair + 1],
                    )

            def emit_k(pair, n8s):
                # NOTE: the k bias is dropped entirely -- q.(Wk x + bk) =
                # q.Wk x + q.bk, and q.bk is constant across keys, so it
                # cancels in the softmax.  The PSUM->SBUF drain is a pure
                # copy and runs on ScalarE (Copy is in every activation
                # table set), rebalancing jp0's drain load off the DVE.
                for n8 in n8s:
                    ps = psA_pool.tile([128, 512], f32, tag="psS")
                    nc.tensor.matmul(
                        ps,
                        lhsT=wt8[:, :, C + pair * 128 : C + (pair + 1) * 128],
                        rhs=x8[:, :, n8 * 512 : (n8 + 1) * 512],
                        start=True,
                        stop=True,
                        perf_mode=DR,
                    )
                    nc.scalar.activation(
                        k2[:, pair, n8 * 512 : (n8 + 1) * 512], ps, AF.Copy
                    )

            def emit_vT(kcps):
                # one [128, 512] PSUM tile + ONE strided drain per chunk
                # PAIR (vs per chunk): halves the DVE drain instruction count
                for kcp in kcps:
                    ps = psA_pool.tile([128, 2, 256], f32, tag="psS")
                    for par in range(2):
                        nc.tensor.matmul(
                            ps[:, par, :],
                            lhsT=x8[:, :, (2 * kcp + par) * 128
                                    : (2 * kcp + par + 1) * 128],
                            rhs=wt8[:, :, 2 * C : 3 * C],
                            start=True,
                            stop=True,
                            perf_mode=DR,
                        )
                    nc.vector.tensor_copy(
                        vT8[:, kcp, :, :, 0:64],
                        ps.rearrange("p x (h d) -> p x h d", h=NH),
                    )

            # minimal prefix before attention starts: one k tile + one q tile
            # (chunks 0-3 only need k-tile 0).  Everything else -- remaining
            # weight-section folds, k tiles, vT chunks, pair-1 qkv -- streams
            # through jp0's deferred slots, paced just ahead of consumption,
            # so the first exp starts ~15us earlier.
            emit_k(0, [0, 1])
            emit_q(0, [0])
            # deferred emission schedule: {jp: {cyc: thunk}}; k tile n8 is
            # consumed from chunk 4*n8, emitted >=5 cycles earlier so its
            # drain never gates a score.
            deferred = {
                0: {
                    0: lambda: (scale_rest(), emit_vT(range(0, 2))),
                    1: lambda: (emit_k(0, [2]), emit_vT(range(2, 4))),
                    4: lambda: (emit_k(0, [3]), emit_vT(range(4, 6))),
                    7: lambda: (emit_k(0, [4]), emit_vT(range(6, 8))),
                    10: lambda: (emit_k(0, [5]), emit_vT(range(8, 10))),
                    13: lambda: (emit_k(0, [6]), emit_vT(range(10, 12))),
                    16: lambda: (emit_k(0, [7]), emit_vT(range(12, 14))),
                    19: lambda: emit_vT(range(14, 16)),
                    22: lambda: emit_k(1, range(0, 4)),
                    25: lambda: emit_k(1, range(4, 8)),
                    28: lambda: emit_q(1, [0]),
                },
                1: {3: lambda: emit_q(0, [1]), 13: lambda: emit_q(1, [1])},
                2: {3: lambda: emit_q(0, [2]), 13: lambda: emit_q(1, [2])},
                3: {3: lambda: emit_q(0, [3]), 13: lambda: emit_q(1, [3])},
            }

            def emit_xres(j):
                # residual fetch for query tile j, issued well before the
                # proj needs it so the DMA latency hides
                xres = work.tile([128, 2, 512], f32, tag="xres")
                nc.sync.dma_start(xres, x_r[:, :, j * 512 : (j + 1) * 512])
                return xres

            def emit_proj(j, att_j, xres):
                # proj + bias + residual for query tile j (emitted lazily so
                # the in-order PE stream never stalls on the division tail)
                y_sb = work.tile([128, 2, 512], f32, tag="y")
                for ot in range(2):
                    psY = psO_pool.tile([128, 512], f32, tag="psO")
                    for h in range(4):
                        nc.tensor.matmul(
                            psY,
                            lhsT=wproj_sb[:, h, ot * 128 : (ot + 1) * 128],
                            rhs=att_j[:, h, :],
                            start=(h == 0),
                            stop=(h == 3),
                        )
                    # y = (psY + projb2) + xres in one DVE op
                    nc.vector.scalar_tensor_tensor(
                        y_sb[:, ot, :], psY, projb2[:, ot : ot + 1],
                        xres[:, ot, :], ALU.add, ALU.add,
                    )
                    # store per output-half: ot0's DMA overlaps ot1's
                    # proj/residual work (shaves the exposed kernel tail)
                    nc.sync.dma_start(
                        y_r[:, ot, j * 512 : (j + 1) * 512], y_sb[:, ot, :]
                    )

            pending = None
            for j in range(4):
                att_j = work.tile([64, NH, 512], bf16, tag="att")
                for pair in range(2):
                    jp = 2 * j + pair
                    dve_kcs = DVE_SCHED.get(jp, DVE_STEADY)
                    defer_jp = deferred.get(jp, {})
                    psO0 = psO_pool.tile([65, 512], f32, tag="psO")
                    psO1 = psO_pool.tile([65, 512], f32, tag="psO")
                    prev_st = None
                    live_P = {}
                    # lag-4 software pipeline on the in-order PE stream:
                    # cycle c emits scores(c) then PV(c-4).  The score tiles
                    # are freed by their exp (not the PV), so the PV lag is
                    # free to be deep -- by lag 4 the exp is always done and
                    # the in-order PE queue never stalls on a PV semaphore.
                    for cyc in range(36):
                        if cyc < 32:
                            kc = cyc
                            # scores for both heads of the pair in one tile:
                            # S[:, 0, :] head 2p (PE rows 0-63), S[:, 1, :]
                            # head 2p+1 (rows 64-127); the two matmuls are
                            # pc-adjacent so they overlap in the PE array.
                            S = (psD_pool if kc % 3 == 2 else psA_pool).tile(
                                [128, 2, 512], f32,
                                tag="psD" if kc % 3 == 2 else "psS",
                            )
                            ma = nc.tensor.matmul(
                                S[:, 0, :],
                                lhsT=k2[0:64, pair, kc * 128 : (kc + 1) * 128],
                                rhs=q2[0:64, pair, j * 512 : (j + 1) * 512],
                                start=True,
                                stop=True,
                            )
                            mb = nc.tensor.matmul(
                                S[:, 1, :],
                                lhsT=k2[64:128, pair, kc * 128 : (kc + 1) * 128],
                                rhs=q2[64:128, pair, j * 512 : (j + 1) * 512],
                                start=True,
                                stop=True,
                            )
                            if prev_st is not None:
                                add_dep_helper(
                                    ma.ins, prev_st, sync=False,
                                    reason="st-pair order",
                                )
                            add_dep_helper(
                                mb.ins, ma.ins, sync=False, reason="st-pair order"
                            )
                            prev_st = mb.ins
                            # exp writes e4m3 into the chunk-pair P tile:
                            # parity 0 (even kc, ScalarE exact exp w/ RNE
                            # cast) or parity 1 (odd kc, VectorE Schraudolph
                            # bits as uint8) -- the engines write disjoint
                            # slices concurrently.
                            if kc % 2 == 0:
                                Pp = pf8.tile([128, 2, 2, 512], f8, tag="Pp")
                                live_P[kc // 2] = Pp
                            else:
                                Pp = live_P[kc // 2]
                            par = kc % 2
                            if kc in dve_kcs:
                                nc.vector.tensor_scalar(
                                    Pp[:, par, :, :].bitcast(u8), S,
                                    A8, B8, ALU.mult, ALU.add,
                                )
                            else:
                                nc.scalar.activation(
                                    Pp[:, par, :, :], S, AF.Exp,
                                    bias=shift_sb, scale=float(HD) ** -0.5,
                                )
                        if cyc >= 5 and (cyc - 5) % 2 == 0:
                            # fp8 DoubleRow PV: one matmul per head covers a
                            # chunk PAIR (contraction 256 = 2 fp8 weights per
                            # PE cell, 0.5 cycles/column) -- halves the PE
                            # time of the PV side.
                            m = (cyc - 5) // 2
                            Pp = live_P.pop(m)
                            for hp, psO in ((0, psO0), (1, psO1)):
                                nc.tensor.matmul(
                                    psO,
                                    lhsT=vT8[:, m, :, 2 * pair + hp, 0:65],
                                    rhs=Pp[:, :, hp, :],
                                    start=(m == 0),
                                    stop=(m == 15),
                                    perf_mode=DR,
                                )
                        if cyc in defer_jp:
                            defer_jp[cyc]()
                        # emit_proj is deferred to cyc 14 of the next pair:
                        # the division chain for att_j (drain -> DMA spread /
                        # recip / broadcast -> gpsimd mul) takes ~10us after
                        # the last PV, and the proj matmuls sit in the
                        # in-order PE stream -- emitted too early they dam up
                        # everything behind them (scores AND the PV-DRs whose
                        # retirement the exp-engine P-tile rotation waits on,
                        # stalling the whole pair ~6.5us).  The xres fetch
                        # has no att dependency and issues at cyc 0.
                        if pending is not None and pair == 0:
                            if cyc == 0:
                                pending = pending + (emit_xres(pending[0]),)
                            elif cyc == 14:
                                emit_proj(*pending)
                                pending = None
                    # copy out of PSUM right away so the psO slots free for
                    # the next head pair; the division chain below works off
                    # the SBUF copy, off the critical path.  Both heads drain
                    # into one [65, 1024] SBUF tile so the reciprocal /
                    # broadcast chain runs once per pair instead of per head.
                    oc = work.tile([65, 1024], f32, tag="oc")
                    nc.scalar.activation(oc[:, 0:512], psO0, AF.Copy)
                    nc.scalar.activation(oc[:, 512:1024], psO1, AF.Copy)
                    # spread the denominator row over 64 partitions so the
                    # iterative DVE reciprocal (8 cyc/elem) is not
                    # single-lane-bound; the broadcast bounces through DRAM
                    # (SBUF DMA sources cannot have a zero partition step,
                    # and GpSimd-side alternatives measured slower).
                    r8 = work.tile([64, 16], f32, tag="r8")
                    nc.sync.dma_start(
                        r8, oc[64:65, :].rearrange("o (a b) -> o a b", b=16)
                    )
                    nc.vector.reciprocal(r8, r8)
                    rec_d = dram_pool.tile([1, 1024], f32, tag="recd")
                    nc.sync.dma_start(
                        rec_d.rearrange("o (a b) -> o a b", b=16), r8
                    )
                    rec_b = work.tile([64, 1024], f32, tag="recb")
                    nc.sync.dma_start(
                        rec_b, rec_d[0:1, :].to_broadcast((64, 1024))
                    )
                    for hp in (0, 1):
                        h = 2 * pair + hp
                        # attention scaling on GpSimd (otherwise idle)
                        nc.gpsimd.tensor_mul(
                            att_j[:, h, :],
                            oc[0:64, hp * 512 : (hp + 1) * 512],
                            rec_b[:, hp * 512 : (hp + 1) * 512],
                        )
                pending = (j, att_j)
            emit_proj(*pending, emit_xres(pending[0]))

    nc.compile()
    return nc


def _get_program():
    if "nc" not in _CACHE:
        _CACHE["nc"] = _build()
    return _CACHE["nc"]


def kernel(x, gn_w, gn_b, qkv_w, qkv_b, proj_w, proj_b):
    import ml_dtypes

    from concourse.bass_utils import run_bass_kernel_spmd

    x = np.asarray(x, np.float32)
    gn_w = np.asarray(gn_w, np.float32)
    gn_b = np.asarray(gn_b, np.float32)
    qkv_w = np.asarray(qkv_w, np.float32)
    qkv_b = np.asarray(qkv_b, np.float32)
    proj_w = np.asarray(proj_w, np.float32)
    proj_b = np.asarray(proj_b, np.float32)

    nc = _get_program()
    qkv_wt = np.ascontiguousarray(qkv_w.T).astype(ml_dtypes.bfloat16)
    proj_wt = np.ascontiguousarray(proj_w.T).astype(ml_dtypes.bfloat16)
    sel = np.zeros((128, 16), np.float32)
    sel[np.arange(128), np.arange(128) // 8] = 1.0
    selT = np.ascontiguousarray(sel.T)

    in_maps = []
    for core in range(8):
        b, half = core // 2, core % 2
        xb = x[b].reshape(C, HW)
        if half == 1:
            xb = np.concatenate([xb[:, NQ:], xb[:, :NQ]], axis=1)
        in_maps.append(
            {
                "x": np.ascontiguousarray(xb),
                "x_bf": np.ascontiguousarray(xb).astype(ml_dtypes.bfloat16),
                "x_f8": np.ascontiguousarray(xb).astype(ml_dtypes.float8_e4m3fn),
                "qkv_wt": qkv_wt,
                "qkv_b": qkv_b,
                "proj_wt": proj_wt,
                "proj_b": proj_b,
                "gn_w": gn_w,
                "gn_b": gn_b,
                "sel": sel,
                "selT": selT,
            }
        )

    res = run_bass_kernel_spmd(nc, in_maps, core_ids=list(range(8)))
    out = np.empty((B, C, HW), np.float32)
    for core in range(8):
        b, half = core // 2, core % 2
        out[b][:, half * NQ : (half + 1) * NQ] = res.results[core]["y"]
    return out.reshape(B, C, 64, 64)

